# revision 1
# baseline (speedup 1.0000x reference)
"""2-layer GCN (GCNConv -> relu -> GCNConv -> sigmoid affine) on TRN2, SPMD over NCORES.

Strategy:
  - Nodes (dst) sharded across cores; edges partitioned by dst shard.
  - Per core, edges sorted into dst-groups of 128, then by src table chunk
    (dma_gather idx is int16 -> gather tables are split into 4 chunks).
  - Aggregation:  aggT[feat, dst128] += msg[e, feat].T @ onehot[e, dst128]
    where msg rows are dma_gather'ed (bf16, dis-prescaled tables) and the
    onehot is built with one DVE tensor_scalar is_equal against an iota row.
  - GCN linearity:  A_hat (x W) == (A_hat x) W, so the dense W matmul runs
    once per 128-dst group on the aggregated tile (fp32).
  - Layer1 output (dis-prescaled, bf16) is AllGather'ed into a full table
    which layer2 gathers from.
"""

import math

import numpy as np
import ml_dtypes

import concourse.bass as bass
import concourse.mybir as mybir
import concourse.tile as tile
from concourse import bacc

P = 128
NCHUNK = 4


# ---------------------------------------------------------------- host side


def make_schedule(dims, seg_len_max):
    """Static (core-independent) schedule.

    seg_len_max: [ngroups, NCHUNK] max-over-cores segment length (edges with
    dst in group g whose table row falls in chunk c).

    Returns dict with per-supergroup call/batch layout.
    """
    ngroups, sg_size = dims["ngroups"], dims["sg_size"]
    pad_len = (np.ceil(seg_len_max / P).astype(np.int64)) * P  # [ngroups, NCHUNK]
    nsg = math.ceil(ngroups / sg_size)
    sgs = []
    slot_off = 0  # slots, across whole layer
    idx_off = 0  # int16 idx columns (16 rows) across whole layer
    batch_off = 0
    # Quantize call lengths so there are few distinct num_idxs values: each
    # distinct value costs one Pool register (48 total on the engine).
    lens = []
    for s in range(nsg):
        groups = list(range(s * sg_size, min((s + 1) * sg_size, ngroups)))
        for c in range(NCHUNK):
            lens.append(int(sum(pad_len[g, c] for g in groups)))
    quant = P
    while len({-(-l // quant) * quant for l in lens if l > 0}) > 16:
        quant *= 2

    for s in range(nsg):
        groups = list(range(s * sg_size, min((s + 1) * sg_size, ngroups)))
        calls = []  # (chunk, num_idxs, idx_col_off_abs, batch_off_in_sg)
        seg_slot = {}  # (g, c) -> slot offset within sg
        sg_slots = 0
        for c in range(NCHUNK):
            call_len = int(sum(pad_len[g, c] for g in groups))
            call_pad = -(-call_len // quant) * quant
            if call_pad > 0:
                calls.append((c, call_pad, idx_off + sg_slots // 16, sg_slots // P))
            for g in groups:
                seg_slot[(g, c)] = sg_slots
                sg_slots += int(pad_len[g, c])
            sg_slots += call_pad - call_len
        gbatches = []  # (g, [batch indices within sg])
        for g in groups:
            bl = []
            for c in range(NCHUNK):
                base = seg_slot[(g, c)] // P
                bl.extend(range(base, base + int(pad_len[g, c]) // P))
            gbatches.append((g, bl))
        sgs.append(
            dict(
                calls=calls,
                groups=gbatches,
                nbatches=sg_slots // P,
                idx_col=idx_off,  # absolute idx col offset of this sg
                idx_ncol=sg_slots // 16,
                batch_off=batch_off,
                slot_off=slot_off,
            )
        )
        slot_off += sg_slots
        idx_off += sg_slots // 16
        batch_off += sg_slots // P
    return dict(
        sgs=sgs,
        total_slots=slot_off,
        total_batches=batch_off,
        max_sg_batches=max(s["nbatches"] for s in sgs),
        pad_len=pad_len,
    )


def fill_core_slots(schedule, core_edges, dims):
    """Build per-core idx (int16 wrapped [16, T/16]) and dl (bf16 [128, B]) arrays.

    core_edges: (g, c, loc, dl) int arrays for this core's edges, any order.
    """
    ngroups = dims["ngroups"]
    g, c, loc, dl = core_edges
    total_slots = schedule["total_slots"]
    idxvals = np.zeros(total_slots, np.int16)
    dlvals = np.full(total_slots, 255.0, np.float32)

    # segment base slots (absolute): recompute from schedule
    seg_base = np.zeros((ngroups, NCHUNK), np.int64)
    for s in schedule["sgs"]:
        off = s["slot_off"]
        pads = schedule["pad_len"]
        for cc in range(NCHUNK):
            for gg, _bl in s["groups"]:
                seg_base[gg, cc] = off
                off += int(pads[gg, cc])

    key = g * NCHUNK + c
    order = np.argsort(key, kind="stable")
    key_s = key[order]
    # rank within segment
    seg_start = np.searchsorted(key_s, np.arange(ngroups * NCHUNK))
    rank = np.arange(len(key_s)) - seg_start[key_s]
    pos = seg_base[g[order], c[order]] + rank
    idxvals[pos] = loc[order].astype(np.int16)
    dlvals[pos] = dl[order]

    wrapped = idxvals.reshape(-1, 16).T  # [16, T/16]; idx i at [i%16, i//16]
    wrapped = np.tile(wrapped, (8, 1)).copy()  # replicated for the 8 Q7 cores
    dltile = dlvals.reshape(-1, P).T.copy()  # [128, B]; slot s at [s%128, s//128]
    return wrapped, dltile


def build_host_data(x, edge_index, W1, b1, W2, b2, ncores=8, sg_size=7):
    N, IN = x.shape
    H = W1.shape[1]
    OUT = W2.shape[1]
    assert N % ncores == 0
    shard = N // ncores
    ngroups = math.ceil(shard / P)
    shard_pad = ngroups * P
    table_rows = shard_pad * ncores
    assert table_rows % NCHUNK == 0
    chunk = table_rows // NCHUNK
    assert chunk - 1 < 2**15, "chunk too large for int16 gather idx"

    dims = dict(
        N=N,
        IN=IN,
        H=H,
        OUT=OUT,
        ncores=ncores,
        shard=shard,
        ngroups=ngroups,
        shard_pad=shard_pad,
        table_rows=table_rows,
        chunk=chunk,
        sg_size=sg_size,
    )

    src = np.concatenate([np.asarray(edge_index[0]), np.arange(N)]).astype(np.int64)
    dst = np.concatenate([np.asarray(edge_index[1]), np.arange(N)]).astype(np.int64)
    deg = np.bincount(dst, minlength=N)
    dis = 1.0 / np.sqrt(np.maximum(deg, 1.0))

    core = dst // shard
    dstloc = dst % shard
    eg = dstloc // P
    edl = (dstloc % P).astype(np.float32)

    # x table: rows in *padded shard* coordinates so that layer1 and layer2
    # tables share the same row mapping (row = shard_pad*(n//shard) + n%shard).
    trow = (src // shard) * shard_pad + (src % shard)
    xt = np.zeros((table_rows, IN), ml_dtypes.bfloat16)
    xs = np.asarray(x, np.float32) * dis[:, None]
    xrow = (np.arange(N) // shard) * shard_pad + (np.arange(N) % shard)
    xt[xrow] = xs.astype(ml_dtypes.bfloat16)

    ec = trow // chunk
    eloc = trow % chunk

    # both layers share the same (g, chunk) structure since table row mapping
    # is identical -> one schedule reused for both layers
    seg_len = np.zeros((ncores, ngroups, NCHUNK), np.int64)
    np.add.at(seg_len, (core, eg, ec), 1)
    schedule = make_schedule(dims, seg_len.max(axis=0))

    per_core = []
    for k in range(ncores):
        m = core == k
        wrapped, dltile = fill_core_slots(
            schedule, (eg[m], ec[m], eloc[m], edl[m]), dims
        )
        disn = np.zeros(shard_pad, np.float32)
        disn[:shard] = dis[k * shard : (k + 1) * shard]
        dis_t = disn.reshape(ngroups, P).T.copy()  # [128, ngroups]
        per_core.append(dict(idx=wrapped, dl=dltile, dis=dis_t))

    consts = dict(
        xt=xt,
        W1=np.asarray(W1, np.float32),
        W2=np.asarray(W2, np.float32),
        b1m=np.tile(np.asarray(b1, np.float32), (P, 1)),
        b2m=np.tile(np.asarray(b2, np.float32), (P, 1)),
        iota=np.tile(np.arange(P, dtype=ml_dtypes.bfloat16), (P, 1)),
    )
    return dims, schedule, consts, per_core


# -------------------------------------------------------------- device side


def build_kernel(nc, dims, schedule, variant="full"):
    dt = mybir.dt
    IN, H, OUT = dims["IN"], dims["H"], dims["OUT"]
    ncores = dims["ncores"]
    table_rows, chunk = dims["table_rows"], dims["chunk"]
    shard_pad = dims["shard_pad"]

    xt = nc.dram_tensor("xt", [table_rows, IN], dt.bfloat16, kind="ExternalInput")
    idx_in = nc.dram_tensor(
        "idx", [P, schedule["total_slots"] // 16], dt.int16, kind="ExternalInput"
    )
    dl_in = nc.dram_tensor(
        "dl", [P, schedule["total_batches"]], dt.float32, kind="ExternalInput"
    )
    dis_in = nc.dram_tensor("dis", [P, dims["ngroups"]], dt.float32, kind="ExternalInput")
    W1_in = nc.dram_tensor("W1", [IN, H], dt.float32, kind="ExternalInput")
    W2_in = nc.dram_tensor("W2", [H, OUT], dt.float32, kind="ExternalInput")
    b1_in = nc.dram_tensor("b1m", [P, H], dt.float32, kind="ExternalInput")
    b2_in = nc.dram_tensor("b2m", [P, OUT], dt.float32, kind="ExternalInput")
    iota_in = nc.dram_tensor("iota", [P, P], dt.bfloat16, kind="ExternalInput")

    h1self = nc.dram_tensor("h1self", [shard_pad, H], dt.bfloat16, kind="Internal")
    h1full = nc.dram_tensor(
        "h1full",
        [table_rows, H],
        dt.bfloat16,
        kind="Internal",
        addr_space="Shared" if ncores > 4 else "Local",
    )
    out = nc.dram_tensor("out", [shard_pad, OUT], dt.float32, kind="ExternalOutput")

    maxb = schedule["max_sg_batches"]

    from concourse.library_config import mlp as mlp_lib

    with tile.TileContext(nc) as tc:
        nc.gpsimd.load_library(mlp_lib)

        # One shared Pool register per distinct gather length (48-reg budget).
        regcache = {}

        def nidx_reg(v):
            if v not in regcache:
                r = nc.gpsimd.alloc_register(f"nidx{v}")
                nc.gpsimd.reg_mov(r, v)
                regcache[v] = r
            return regcache[v]
        with (
            tc.tile_pool(name="const", bufs=1) as cpool,
            tc.tile_pool(name="gather", bufs=2) as gpool,
            tc.tile_pool(name="meta", bufs=2) as mpool,
            tc.tile_pool(name="oh", bufs=4) as ohpool,
            tc.tile_pool(name="ep", bufs=3) as epool,
            tc.tile_pool(name="aggp", bufs=2, space="PSUM") as aggpool,
            tc.tile_pool(name="densep", bufs=2, space="PSUM") as dpool,
        ):
            W1s = cpool.tile([IN, H], dt.float32)
            W2s = cpool.tile([H, OUT], dt.float32)
            b1s = cpool.tile([P, H], dt.float32)
            b2s = cpool.tile([P, OUT], dt.float32)
            iotas = cpool.tile([P, P], dt.bfloat16)
            diss = cpool.tile([P, dims["ngroups"]], dt.float32)
            nc.sync.dma_start(out=W1s[:], in_=W1_in[:, :])
            nc.sync.dma_start(out=W2s[:], in_=W2_in[:, :])
            nc.sync.dma_start(out=b1s[:], in_=b1_in[:, :])
            nc.sync.dma_start(out=b2s[:], in_=b2_in[:, :])
            nc.sync.dma_start(out=iotas[:], in_=iota_in[:, :])
            nc.sync.dma_start(out=diss[:], in_=dis_in[:, :])

            layers = (0,) if variant == "layer1" else (0, 1)
            for layer in layers:
                table = xt if layer == 0 else h1full
                HH = H if layer == 0 else OUT
                Wt = W1s if layer == 0 else W2s
                bt = b1s if layer == 0 else b2s

                for s in schedule["sgs"]:
                    gtile = gpool.tile([P, maxb * P], dt.bfloat16, tag="g")
                    itile = mpool.tile(
                        [P, schedule["max_sg_batches"] * 8], dt.int16, tag="i"
                    )
                    dtile = mpool.tile([P, maxb], dt.float32, tag="d")
                    nc.sync.dma_start(
                        out=itile[:, : s["idx_ncol"]],
                        in_=idx_in[:, s["idx_col"] : s["idx_col"] + s["idx_ncol"]],
                    )
                    nc.sync.dma_start(
                        out=dtile[:, : s["nbatches"]],
                        in_=dl_in[:, s["batch_off"] : s["batch_off"] + s["nbatches"]],
                    )
                    for cnum, clen, coff, boff in s["calls"]:
                        nc.gpsimd.dma_gather(
                            out_ap=gtile[:, boff * P : boff * P + clen].rearrange(
                                "p (b f) -> p b f", f=P
                            ),
                            in_ap=table[cnum * chunk : (cnum + 1) * chunk, :],
                            idxs_ap=itile[:, coff - s["idx_col"] : coff - s["idx_col"] + clen // 16],
                            num_idxs=clen,
                            num_idxs_reg=nidx_reg(clen),
                            elem_size=IN if layer == 0 else H,
                            single_packet=False,
                        )
                    for gg, bl in s["groups"]:
                        agg = aggpool.tile([P, P], dt.float32, tag="agg")
                        for j, b in enumerate(bl):
                            oh = ohpool.tile([P, P], dt.bfloat16, tag="oh")
                            nc.vector.tensor_scalar(
                                out=oh[:],
                                in0=iotas[:],
                                scalar1=dtile[:, b : b + 1],
                                scalar2=None,
                                op0=mybir.AluOpType.is_equal,
                            )
                            nc.tensor.matmul(
                                out=agg[:],
                                lhsT=gtile[:, b * P : (b + 1) * P],
                                rhs=oh[:],
                                start=(j == 0),
                                stop=(j == len(bl) - 1),
                            )
                        aggs = epool.tile([P, P], dt.float32, tag="aggs")
                        nc.vector.tensor_copy(out=aggs[:], in_=agg[:])
                        hraw = dpool.tile([P, HH], dt.float32, tag="hraw")
                        nc.tensor.matmul(
                            out=hraw[:], lhsT=aggs[:], rhs=Wt[:], start=True, stop=True
                        )
                        t1 = epool.tile([P, HH], dt.float32, tag="t1")
                        nc.vector.tensor_scalar(
                            out=t1[:],
                            in0=hraw[:],
                            scalar1=diss[:, gg : gg + 1],
                            scalar2=None,
                            op0=mybir.AluOpType.mult,
                        )
                        nc.vector.tensor_tensor(
                            out=t1[:], in0=t1[:], in1=bt[:], op=mybir.AluOpType.add
                        )
                        if layer == 0:
                            t2 = epool.tile([P, HH], dt.float32, tag="t2")
                            nc.scalar.activation(
                                out=t2[:], in_=t1[:], func=mybir.ActivationFunctionType.Relu
                            )
                            hst = epool.tile([P, HH], dt.bfloat16, tag="hst")
                            nc.vector.tensor_scalar(
                                out=hst[:],
                                in0=t2[:],
                                scalar1=diss[:, gg : gg + 1],
                                scalar2=None,
                                op0=mybir.AluOpType.mult,
                            )
                            nc.sync.dma_start(
                                out=h1self[gg * P : (gg + 1) * P, :], in_=hst[:]
                            )
                        else:
                            t2 = epool.tile([P, HH], dt.float32, tag="t2")
                            nc.scalar.activation(
                                out=t2[:],
                                in_=t1[:],
                                func=mybir.ActivationFunctionType.Sigmoid,
                            )
                            ot = epool.tile([P, HH], dt.float32, tag="ot")
                            nc.vector.tensor_scalar(
                                out=ot[:],
                                in0=t2[:],
                                scalar1=0.8,
                                scalar2=0.1,
                                op0=mybir.AluOpType.mult,
                                op1=mybir.AluOpType.add,
                            )
                            nc.sync.dma_start(
                                out=out[gg * P : (gg + 1) * P, :], in_=ot[:]
                            )
                if layer == 0 and variant == "full":
                    nc.gpsimd.collective_compute(
                        kind="AllGather",
                        op=mybir.AluOpType.bypass,
                        replica_groups=[list(range(ncores))],
                        ins=[h1self[:, :]],
                        outs=[h1full[:, :]],
                    )
                elif layer == 0 and variant == "nocoll":
                    nc.sync.dma_start(out=h1full[:shard_pad, :], in_=h1self[:, :])
    return nc


def make_in_maps(dims, consts, per_core):
    in_maps = []
    for pc in per_core:
        in_maps.append(
            dict(
                xt=consts["xt"],
                idx=pc["idx"],
                dl=pc["dl"],
                dis=pc["dis"],
                W1=consts["W1"],
                W2=consts["W2"],
                b1m=consts["b1m"],
                b2m=consts["b2m"],
                iota=consts["iota"],
            )
        )
    return in_maps


def _install_ntff_hook():
    """Provide antenv.axon_hooks (missing on this image) so that
    run_bass_kernel_spmd(trace=True) can capture NTFF profiles via the
    axon .so's NRT-profile C ABI."""
    import sys
    import types

    if "antenv.axon_hooks" in sys.modules:
        return
    try:
        import antenv
        from trn_agent_boot.trn_boot import _ntff_profile_via_ctypes

        hook = _ntff_profile_via_ctypes("/opt/axon/libaxon_pjrt.so")
        mod = types.ModuleType("antenv.axon_hooks")
        mod._hook = hook

        def get_axon_ntff_profile_hook():
            return mod._hook

        def set_axon_ntff_profile_hook(h):
            mod._hook = h

        mod.get_axon_ntff_profile_hook = get_axon_ntff_profile_hook
        mod.set_axon_ntff_profile_hook = set_axon_ntff_profile_hook
        sys.modules["antenv.axon_hooks"] = mod
        antenv.axon_hooks = mod
    except Exception as e:  # pragma: no cover
        print("ntff hook install failed:", e)


def run(x, edge_index, W1, b1, W2, b2, ncores=8, sg_size=7, trace=False, variant="full"):
    from concourse import bass_utils

    if trace:
        _install_ntff_hook()

    dims, schedule, consts, per_core = build_host_data(
        x, edge_index, W1, b1, W2, b2, ncores=ncores, sg_size=sg_size
    )
    nc = bacc.Bacc(num_devices=ncores)
    build_kernel(nc, dims, schedule, variant=variant)
    nc.compile()
    in_maps = make_in_maps(dims, consts, per_core)
    res = bass_utils.run_bass_kernel_spmd(
        nc, in_maps, core_ids=list(range(ncores)), trace=trace
    )
    shard, shard_pad = dims["shard"], dims["shard_pad"]
    full = np.concatenate([r["out"][:shard] for r in res.results], axis=0)
    return full, res


# ------------------------------------------------------------- harness entry


def kernel(**inputs):
    """Full (unsharded) inputs -> full output, computed on 8 NeuronCores."""
    out, _ = run(
        np.asarray(inputs["x"], np.float32),
        np.asarray(inputs["edge_index"]),
        np.asarray(inputs["W1"], np.float32),
        np.asarray(inputs["b1"], np.float32),
        np.asarray(inputs["W2"], np.float32),
        np.asarray(inputs["b2"], np.float32),
        ncores=8,
        sg_size=7,
        trace=False,
    )
    return out.astype(np.float32)



# revision 9
# speedup vs baseline: 1.4324x; 1.4324x over previous
"""2-layer GCN (GCNConv -> relu -> GCNConv -> sigmoid affine) on TRN2, SPMD over NCORES.

Strategy:
  - Nodes (dst) sharded across cores; edges partitioned by dst shard.
  - Per core, edges sorted into dst-groups of 128, then by src table chunk
    (dma_gather idx is int16 -> gather tables are split into 4 chunks).
  - Aggregation:  aggT[feat, dst128] += msg[e, feat].T @ onehot[e, dst128]
    where msg rows are dma_gather'ed (bf16, dis-prescaled tables) and the
    onehot is built with one DVE tensor_scalar is_equal against an iota row.
  - GCN linearity:  A_hat (x W) == (A_hat x) W, so the dense W matmul runs
    once per 128-dst group on the aggregated tile (fp32).
  - Layer1 output (dis-prescaled, bf16) is AllGather'ed into a full table
    which layer2 gathers from.
"""

import math

import numpy as np
import ml_dtypes

import concourse.bass as bass
import concourse.mybir as mybir
import concourse.tile as tile
from concourse import bacc

P = 128
NCHUNK = 4


# ---------------------------------------------------------------- host side


def make_schedule(dims, seg_len_max):
    """Static (core-independent) schedule.

    seg_len_max: [ngroups, NCHUNK] max-over-cores segment length (edges with
    dst in group g whose table row falls in chunk c).

    Returns dict with per-supergroup call/batch layout.
    """
    ngroups, sg_size = dims["ngroups"], dims["sg_size"]
    pad_len = (np.ceil(seg_len_max / P).astype(np.int64)) * P  # [ngroups, NCHUNK]
    nsg = math.ceil(ngroups / sg_size)
    sgs = []
    slot_off = 0  # slots, across whole layer
    idx_off = 0  # int16 idx columns (16 rows) across whole layer
    batch_off = 0
    # Quantize call lengths so there are few distinct num_idxs values: each
    # distinct value costs one Pool register (48 total on the engine).
    lens = []
    for s in range(nsg):
        groups = list(range(s * sg_size, min((s + 1) * sg_size, ngroups)))
        for c in range(NCHUNK):
            lens.append(int(sum(pad_len[g, c] for g in groups)))
    quant = P
    while len({-(-l // quant) * quant for l in lens if l > 0}) > 16:
        quant *= 2

    for s in range(nsg):
        groups = list(range(s * sg_size, min((s + 1) * sg_size, ngroups)))
        calls = []  # (chunk, num_idxs, idx_col_off_abs, batch_off_in_sg)
        seg_slot = {}  # (g, c) -> slot offset within sg
        sg_slots = 0
        for c in range(NCHUNK):
            call_len = int(sum(pad_len[g, c] for g in groups))
            call_pad = -(-call_len // quant) * quant
            if call_pad > 0:
                calls.append((c, call_pad, idx_off + sg_slots // 16, sg_slots // P))
            for g in groups:
                seg_slot[(g, c)] = sg_slots
                sg_slots += int(pad_len[g, c])
            sg_slots += call_pad - call_len
        gbatches = []  # (g, [batch indices within sg])
        for g in groups:
            bl = []
            for c in range(NCHUNK):
                base = seg_slot[(g, c)] // P
                bl.extend(range(base, base + int(pad_len[g, c]) // P))
            gbatches.append((g, bl))
        sgs.append(
            dict(
                calls=calls,
                groups=gbatches,
                nbatches=sg_slots // P,
                idx_col=idx_off,  # absolute idx col offset of this sg
                idx_ncol=sg_slots // 16,
                batch_off=batch_off,
                slot_off=slot_off,
            )
        )
        slot_off += sg_slots
        idx_off += sg_slots // 16
        batch_off += sg_slots // P
    return dict(
        sgs=sgs,
        total_slots=slot_off,
        total_batches=batch_off,
        max_sg_batches=max(s["nbatches"] for s in sgs),
        pad_len=pad_len,
    )


def fill_core_slots(schedule, core_edges, dims):
    """Build per-core idx (int16 wrapped [16, T/16]) and dl (bf16 [128, B]) arrays.

    core_edges: (g, c, loc, dl) int arrays for this core's edges, any order.
    """
    ngroups = dims["ngroups"]
    g, c, loc, dl = core_edges
    total_slots = schedule["total_slots"]
    idxvals = np.zeros(total_slots, np.int16)
    dlvals = np.full(total_slots, 255.0, np.float32)  # 255 -> all-zero onehot col

    # segment base slots (absolute): recompute from schedule
    seg_base = np.zeros((ngroups, NCHUNK), np.int64)
    for s in schedule["sgs"]:
        off = s["slot_off"]
        pads = schedule["pad_len"]
        for cc in range(NCHUNK):
            for gg, _bl in s["groups"]:
                seg_base[gg, cc] = off
                off += int(pads[gg, cc])

    key = g * NCHUNK + c
    order = np.argsort(key, kind="stable")
    key_s = key[order]
    # rank within segment
    seg_start = np.searchsorted(key_s, np.arange(ngroups * NCHUNK))
    rank = np.arange(len(key_s)) - seg_start[key_s]
    pos = seg_base[g[order], c[order]] + rank
    idxvals[pos] = loc[order].astype(np.int16)
    dlvals[pos] = dl[order]

    wrapped = idxvals.reshape(-1, 16).T  # [16, T/16]; idx i at [i%16, i//16]
    wrapped = np.tile(wrapped, (8, 1)).copy()  # replicated for the 8 Q7 cores
    # [128, B]; slot s at [s%128, s//128]; bf16 exact for ints <= 255
    dltile = dlvals.reshape(-1, P).T.astype(ml_dtypes.bfloat16).copy()
    return wrapped, dltile


def build_host_data(x, edge_index, W1, b1, W2, b2, ncores=8, sg_size=7):
    N, IN = x.shape
    H = W1.shape[1]
    OUT = W2.shape[1]
    assert N % ncores == 0
    shard = N // ncores
    ngroups = math.ceil(shard / P)
    shard_pad = ngroups * P
    table_rows = shard_pad * ncores
    assert table_rows % NCHUNK == 0
    chunk = table_rows // NCHUNK
    assert chunk - 1 < 2**15, "chunk too large for int16 gather idx"

    dims = dict(
        N=N,
        IN=IN,
        H=H,
        OUT=OUT,
        ncores=ncores,
        shard=shard,
        ngroups=ngroups,
        shard_pad=shard_pad,
        table_rows=table_rows,
        chunk=chunk,
        sg_size=sg_size,
    )

    src = np.concatenate([np.asarray(edge_index[0]), np.arange(N)]).astype(np.int64)
    dst = np.concatenate([np.asarray(edge_index[1]), np.arange(N)]).astype(np.int64)
    deg = np.bincount(dst, minlength=N)
    dis = 1.0 / np.sqrt(np.maximum(deg, 1.0))

    core = dst // shard
    dstloc = dst % shard
    eg = dstloc // P
    edl = (dstloc % P).astype(np.float32)

    # x table: rows in *padded shard* coordinates so that layer1 and layer2
    # tables share the same row mapping (row = shard_pad*(n//shard) + n%shard).
    trow = (src // shard) * shard_pad + (src % shard)
    xt = np.zeros((table_rows, IN), ml_dtypes.bfloat16)
    xs = np.asarray(x, np.float32) * dis[:, None]
    xrow = (np.arange(N) // shard) * shard_pad + (np.arange(N) % shard)
    xt[xrow] = xs.astype(ml_dtypes.bfloat16)

    ec = trow // chunk
    eloc = trow % chunk

    # both layers share the same (g, chunk) structure since table row mapping
    # is identical -> one schedule reused for both layers
    seg_len = np.zeros((ncores, ngroups, NCHUNK), np.int64)
    np.add.at(seg_len, (core, eg, ec), 1)
    schedule = make_schedule(dims, seg_len.max(axis=0))

    per_core = []
    for k in range(ncores):
        m = core == k
        wrapped, dltile = fill_core_slots(
            schedule, (eg[m], ec[m], eloc[m], edl[m]), dims
        )
        disn = np.zeros(shard_pad, np.float32)
        disn[:shard] = dis[k * shard : (k + 1) * shard]
        dis_t = disn.reshape(ngroups, P).T.copy()  # [128, ngroups]
        per_core.append(dict(idx=wrapped, dl=dltile, dis=dis_t))

    consts = dict(
        xt=xt,
        W1=np.asarray(W1, np.float32),
        W2=np.asarray(W2, np.float32),
        b1m=np.tile(np.asarray(b1, np.float32), (P, 1)),
        b2m=np.tile(np.asarray(b2, np.float32), (P, 1)),
        iota=np.tile(np.arange(P, dtype=ml_dtypes.bfloat16), (P, 1)),
    )
    return dims, schedule, consts, per_core


# -------------------------------------------------------------- device side


def build_kernel(nc, dims, schedule, variant="full"):
    dt = mybir.dt
    IN, H, OUT = dims["IN"], dims["H"], dims["OUT"]
    ncores = dims["ncores"]
    table_rows, chunk = dims["table_rows"], dims["chunk"]
    shard_pad = dims["shard_pad"]

    xt = nc.dram_tensor("xt", [table_rows, IN], dt.bfloat16, kind="ExternalInput")
    idx_in = nc.dram_tensor(
        "idx", [P, schedule["total_slots"] // 16], dt.int16, kind="ExternalInput"
    )
    dl_in = nc.dram_tensor(
        "dl", [P, schedule["total_batches"]], dt.bfloat16, kind="ExternalInput"
    )
    dis_in = nc.dram_tensor("dis", [P, dims["ngroups"]], dt.float32, kind="ExternalInput")
    W1_in = nc.dram_tensor("W1", [IN, H], dt.float32, kind="ExternalInput")
    W2_in = nc.dram_tensor("W2", [H, OUT], dt.float32, kind="ExternalInput")
    b1_in = nc.dram_tensor("b1m", [P, H], dt.float32, kind="ExternalInput")
    b2_in = nc.dram_tensor("b2m", [P, OUT], dt.float32, kind="ExternalInput")
    iota_in = nc.dram_tensor("iota", [P, P], dt.bfloat16, kind="ExternalInput")

    h1self = nc.dram_tensor("h1self", [shard_pad, H], dt.bfloat16, kind="Internal")
    h1full = nc.dram_tensor(
        "h1full",
        [table_rows, H],
        dt.bfloat16,
        kind="Internal",
        addr_space="Shared" if ncores > 4 else "Local",
    )
    out = nc.dram_tensor("out", [shard_pad, OUT], dt.float32, kind="ExternalOutput")

    maxb = schedule["max_sg_batches"]

    from concourse.library_config import mlp as mlp_lib

    with tile.TileContext(nc) as tc:
        nc.gpsimd.load_library(mlp_lib)

        # One shared Pool register per distinct gather length (48-reg budget).
        regcache = {}

        def nidx_reg(v):
            if v not in regcache:
                r = nc.gpsimd.alloc_register(f"nidx{v}")
                nc.gpsimd.reg_mov(r, v)
                regcache[v] = r
            return regcache[v]
        with (
            tc.tile_pool(name="const", bufs=1) as cpool,
            tc.tile_pool(name="gather", bufs=2) as gpool,
            tc.tile_pool(name="meta", bufs=2) as mpool,
            tc.tile_pool(name="oh", bufs=2) as ohpool,
            tc.tile_pool(name="ep", bufs=3) as epool,
            tc.tile_pool(name="aggp", bufs=2, space="PSUM") as aggpool,
            tc.tile_pool(name="densep", bufs=2, space="PSUM") as dpool,
        ):
            W1s = cpool.tile([IN, H], dt.float32)
            W2s = cpool.tile([H, OUT], dt.float32)
            b1s = cpool.tile([P, H], dt.float32)
            b2s = cpool.tile([P, OUT], dt.float32)
            iotas = cpool.tile([P, P], dt.bfloat16)
            diss = cpool.tile([P, dims["ngroups"]], dt.float32)
            nc.sync.dma_start(out=W1s[:], in_=W1_in[:, :])
            nc.sync.dma_start(out=W2s[:], in_=W2_in[:, :])
            nc.sync.dma_start(out=b1s[:], in_=b1_in[:, :])
            nc.sync.dma_start(out=b2s[:], in_=b2_in[:, :])
            nc.sync.dma_start(out=iotas[:], in_=iota_in[:, :])
            nc.sync.dma_start(out=diss[:], in_=dis_in[:, :])

            c01 = cpool.tile([P, 1], dt.float32)
            nc.vector.memset(c01[:], 0.1)

            # Tile assigns SWDGE completion sems round-robin over 8 lanes
            # (one per Pool-DMA instruction, in program order) and each sem
            # is locked to a single SWDGE queue.  Derive queue from the same
            # mod-8 counter so lane L always sees one queue.  Queues 1-3
            # only: queue 0's Q7 pair includes core 0, which gates
            # instruction dispatch for the whole cluster.
            qmap = [1, 2, 3, 1, 2, 3, 1, 2]
            qi = 0

            layers = (0,) if variant == "layer1" else (0, 1)
            for layer in layers:
                table = xt if layer == 0 else h1full
                HH = H if layer == 0 else OUT
                Wt = W1s if layer == 0 else W2s
                bt = b1s if layer == 0 else b2s

                for s in schedule["sgs"]:
                    nb = s["nbatches"]
                    gtile = gpool.tile([P, maxb * P], dt.bfloat16, tag="g")
                    itile = mpool.tile(
                        [P, schedule["max_sg_batches"] * 8], dt.int16, tag="i"
                    )
                    dtile = mpool.tile([P, maxb], dt.bfloat16, tag="d")
                    nc.sync.dma_start(
                        out=itile[:, : s["idx_ncol"]],
                        in_=idx_in[:, s["idx_col"] : s["idx_col"] + s["idx_ncol"]],
                    )
                    nc.sync.dma_start(
                        out=dtile[:, :nb],
                        in_=dl_in[:, s["batch_off"] : s["batch_off"] + nb],
                    )
                    for cnum, clen, coff, boff in s["calls"]:
                        nc.gpsimd.dma_gather(
                            out_ap=gtile[:, boff * P : boff * P + clen].rearrange(
                                "p (b f) -> p b f", f=P
                            ),
                            in_ap=table[cnum * chunk : (cnum + 1) * chunk, :],
                            idxs_ap=itile[:, coff - s["idx_col"] : coff - s["idx_col"] + clen // 16],
                            num_idxs=clen,
                            num_idxs_reg=nidx_reg(clen),
                            elem_size=IN if layer == 0 else H,
                            single_packet=False,
                            queue_num=qmap[qi % 8],
                        )
                        qi += 1
                    # One-hot matrix for every batch of this sg in one DVE op:
                    # oh[p, b, j] = (dl[p, b] == j).  tensor_tensor runs in
                    # single-port mode so it never locks GpSimd out of the
                    # shared SBUF port pair (tensor_scalar would).
                    ohtile = ohpool.tile([P, maxb * P], dt.bfloat16, tag="oh")
                    nc.vector.tensor_tensor(
                        out=ohtile[:, : nb * P].rearrange("p (b f) -> p b f", f=P),
                        in0=dtile[:, :nb].unsqueeze(2).broadcast_to([P, nb, P]),
                        in1=iotas[:, :].unsqueeze(1).broadcast_to([P, nb, P]),
                        op=mybir.AluOpType.is_equal,
                    )
                    for gg, bl in s["groups"]:
                        agg = aggpool.tile([P, P], dt.float32, tag="agg")
                        for j, b in enumerate(bl):
                            nc.tensor.matmul(
                                out=agg[:],
                                lhsT=gtile[:, b * P : (b + 1) * P],
                                rhs=ohtile[:, b * P : (b + 1) * P],
                                start=(j == 0),
                                stop=(j == len(bl) - 1),
                            )
                        aggs = epool.tile([P, P], dt.float32, tag="aggs")
                        nc.vector.tensor_copy(out=aggs[:], in_=agg[:])
                        hraw = dpool.tile([P, HH], dt.float32, tag="hraw")
                        nc.tensor.matmul(
                            out=hraw[:], lhsT=aggs[:], rhs=Wt[:], start=True, stop=True
                        )
                        t1 = epool.tile([P, HH], dt.float32, tag="t1")
                        nc.vector.scalar_tensor_tensor(
                            out=t1[:],
                            in0=hraw[:],
                            scalar=diss[:, gg : gg + 1],
                            in1=bt[:],
                            op0=mybir.AluOpType.mult,
                            op1=mybir.AluOpType.add,
                        )
                        if layer == 0:
                            t2 = epool.tile([P, HH], dt.float32, tag="t2")
                            nc.scalar.activation(
                                out=t2[:], in_=t1[:], func=mybir.ActivationFunctionType.Relu
                            )
                            hst = epool.tile([P, HH], dt.bfloat16, tag="hst")
                            nc.vector.tensor_tensor(
                                out=hst[:],
                                in0=t2[:],
                                in1=diss[:, gg : gg + 1].broadcast_to([P, HH]),
                                op=mybir.AluOpType.mult,
                            )
                            nc.sync.dma_start(
                                out=h1self[gg * P : (gg + 1) * P, :], in_=hst[:]
                            )
                        else:
                            t2 = epool.tile([P, HH], dt.float32, tag="t2")
                            nc.scalar.activation(
                                out=t2[:],
                                in_=t1[:],
                                func=mybir.ActivationFunctionType.Sigmoid,
                            )
                            ot = epool.tile([P, HH], dt.float32, tag="ot")
                            nc.vector.scalar_tensor_tensor(
                                out=ot[:],
                                in0=t2[:],
                                scalar=0.8,
                                in1=c01[:, 0:1].broadcast_to([P, HH]),
                                op0=mybir.AluOpType.mult,
                                op1=mybir.AluOpType.add,
                            )
                            nc.sync.dma_start(
                                out=out[gg * P : (gg + 1) * P, :], in_=ot[:]
                            )
                if layer == 0 and variant == "full":
                    nc.gpsimd.collective_compute(
                        kind="AllGather",
                        op=mybir.AluOpType.bypass,
                        replica_groups=[list(range(ncores))],
                        ins=[h1self[:, :]],
                        outs=[h1full[:, :]],
                    )
                elif layer == 0 and variant == "nocoll":
                    nc.sync.dma_start(out=h1full[:shard_pad, :], in_=h1self[:, :])
    return nc


def make_in_maps(dims, consts, per_core):
    in_maps = []
    for pc in per_core:
        in_maps.append(
            dict(
                xt=consts["xt"],
                idx=pc["idx"],
                dl=pc["dl"],
                dis=pc["dis"],
                W1=consts["W1"],
                W2=consts["W2"],
                b1m=consts["b1m"],
                b2m=consts["b2m"],
                iota=consts["iota"],
            )
        )
    return in_maps


def _install_ntff_hook():
    """Provide antenv.axon_hooks (missing on this image) so that
    run_bass_kernel_spmd(trace=True) can capture NTFF profiles via the
    axon .so's NRT-profile C ABI."""
    import sys
    import types

    if "antenv.axon_hooks" in sys.modules:
        return
    try:
        import antenv
        from trn_agent_boot.trn_boot import _ntff_profile_via_ctypes

        hook = _ntff_profile_via_ctypes("/opt/axon/libaxon_pjrt.so")
        mod = types.ModuleType("antenv.axon_hooks")
        mod._hook = hook

        def get_axon_ntff_profile_hook():
            return mod._hook

        def set_axon_ntff_profile_hook(h):
            mod._hook = h

        mod.get_axon_ntff_profile_hook = get_axon_ntff_profile_hook
        mod.set_axon_ntff_profile_hook = set_axon_ntff_profile_hook
        sys.modules["antenv.axon_hooks"] = mod
        antenv.axon_hooks = mod
    except Exception as e:  # pragma: no cover
        print("ntff hook install failed:", e)


def run(x, edge_index, W1, b1, W2, b2, ncores=8, sg_size=7, trace=False, variant="full"):
    from concourse import bass_utils

    if trace:
        _install_ntff_hook()

    dims, schedule, consts, per_core = build_host_data(
        x, edge_index, W1, b1, W2, b2, ncores=ncores, sg_size=sg_size
    )
    nc = bacc.Bacc(num_devices=ncores, num_swdge_queues=4)
    build_kernel(nc, dims, schedule, variant=variant)
    nc.compile()
    in_maps = make_in_maps(dims, consts, per_core)
    res = bass_utils.run_bass_kernel_spmd(
        nc, in_maps, core_ids=list(range(ncores)), trace=trace
    )
    shard, shard_pad = dims["shard"], dims["shard_pad"]
    full = np.concatenate([r["out"][:shard] for r in res.results], axis=0)
    return full, res


# ------------------------------------------------------------- harness entry


def kernel(**inputs):
    """Full (unsharded) inputs -> full output, computed on 8 NeuronCores."""
    out, _ = run(
        np.asarray(inputs["x"], np.float32),
        np.asarray(inputs["edge_index"]),
        np.asarray(inputs["W1"], np.float32),
        np.asarray(inputs["b1"], np.float32),
        np.asarray(inputs["W2"], np.float32),
        np.asarray(inputs["b2"], np.float32),
        ncores=8,
        sg_size=7,
        trace=False,
    )
    return out.astype(np.float32)



# revision 13
# speedup vs baseline: 1.8063x; 1.2610x over previous
"""2-layer GCN (GCNConv -> relu -> GCNConv -> sigmoid affine) on TRN2, SPMD over NCORES.

Strategy:
  - Nodes (dst) sharded across cores; edges partitioned by dst shard.
  - Per core, edges sorted into dst-groups of 128, then by src table chunk
    (dma_gather idx is int16 -> gather tables are split into 4 chunks).
  - Aggregation:  aggT[feat, dst128] += msg[e, feat].T @ onehot[e, dst128]
    where msg rows are dma_gather'ed (bf16, dis-prescaled tables) and the
    onehot is built with one DVE tensor_scalar is_equal against an iota row.
  - GCN linearity:  A_hat (x W) == (A_hat x) W, so the dense W matmul runs
    once per 128-dst group on the aggregated tile (fp32).
  - Layer1 output (dis-prescaled, bf16) is AllGather'ed into a full table
    which layer2 gathers from.
"""

import math

import numpy as np
import ml_dtypes

import concourse.bass as bass
import concourse.mybir as mybir
import concourse.tile as tile
from concourse import bacc

P = 128
NCHUNK = 4


# ---------------------------------------------------------------- host side


def make_schedule(dims, seg_len_max):
    """Static (core-independent) schedule.

    seg_len_max: [ngroups, NCHUNK] max-over-cores segment length (edges with
    dst in group g whose table row falls in chunk c).

    Returns dict with per-supergroup call/batch layout.
    """
    ngroups, sg_size = dims["ngroups"], dims["sg_size"]
    pad_len = (np.ceil(seg_len_max / P).astype(np.int64)) * P  # [ngroups, NCHUNK]
    nsg = math.ceil(ngroups / sg_size)
    sgs = []
    slot_off = 0  # slots, across whole layer
    idx_off = 0  # int16 idx columns (16 rows) across whole layer
    batch_off = 0
    # Quantize call lengths so there are few distinct num_idxs values: each
    # distinct value costs one Pool register (48 total on the engine).
    lens = []
    for s in range(nsg):
        groups = list(range(s * sg_size, min((s + 1) * sg_size, ngroups)))
        for c in range(NCHUNK):
            lens.append(int(sum(pad_len[g, c] for g in groups)))
    quant = P
    while len({-(-l // quant) * quant for l in lens if l > 0}) > 16:
        quant *= 2

    for s in range(nsg):
        groups = list(range(s * sg_size, min((s + 1) * sg_size, ngroups)))
        calls = []  # (chunk, num_idxs, idx_col_off_abs, batch_off_in_sg)
        seg_slot = {}  # (g, c) -> slot offset within sg
        sg_slots = 0
        for c in range(NCHUNK):
            call_len = int(sum(pad_len[g, c] for g in groups))
            call_pad = -(-call_len // quant) * quant
            if call_pad > 0:
                calls.append((c, call_pad, idx_off + sg_slots // 16, sg_slots // P))
            for g in groups:
                seg_slot[(g, c)] = sg_slots
                sg_slots += int(pad_len[g, c])
            sg_slots += call_pad - call_len
        gbatches = []  # (g, [batch indices within sg])
        for g in groups:
            bl = []
            for c in range(NCHUNK):
                base = seg_slot[(g, c)] // P
                bl.extend(range(base, base + int(pad_len[g, c]) // P))
            gbatches.append((g, bl))
        sgs.append(
            dict(
                calls=calls,
                groups=gbatches,
                nbatches=sg_slots // P,
                idx_col=idx_off,  # absolute idx col offset of this sg
                idx_ncol=sg_slots // 16,
                batch_off=batch_off,
                slot_off=slot_off,
            )
        )
        slot_off += sg_slots
        idx_off += sg_slots // 16
        batch_off += sg_slots // P
    return dict(
        sgs=sgs,
        total_slots=slot_off,
        total_batches=batch_off,
        max_sg_batches=max(s["nbatches"] for s in sgs),
        pad_len=pad_len,
    )


def fill_core_slots(schedule, core_edges, dims):
    """Build per-core idx (int16 wrapped [16, T/16]) and dl (bf16 [128, B]) arrays.

    core_edges: (g, c, loc, dl) int arrays for this core's edges, any order.
    """
    ngroups = dims["ngroups"]
    g, c, loc, dl = core_edges
    total_slots = schedule["total_slots"]
    idxvals = np.zeros(total_slots, np.int16)
    dlvals = np.full(total_slots, 255.0, np.float32)  # 255 -> all-zero onehot col

    # segment base slots (absolute): recompute from schedule
    seg_base = np.zeros((ngroups, NCHUNK), np.int64)
    for s in schedule["sgs"]:
        off = s["slot_off"]
        pads = schedule["pad_len"]
        for cc in range(NCHUNK):
            for gg, _bl in s["groups"]:
                seg_base[gg, cc] = off
                off += int(pads[gg, cc])

    key = g * NCHUNK + c
    order = np.argsort(key, kind="stable")
    key_s = key[order]
    # rank within segment
    seg_start = np.searchsorted(key_s, np.arange(ngroups * NCHUNK))
    rank = np.arange(len(key_s)) - seg_start[key_s]
    pos = seg_base[g[order], c[order]] + rank
    idxvals[pos] = loc[order].astype(np.int16)
    dlvals[pos] = dl[order]

    wrapped = idxvals.reshape(-1, 16).T  # [16, T/16]; idx i at [i%16, i//16]
    wrapped = np.tile(wrapped, (8, 1)).copy()  # replicated for the 8 Q7 cores
    # [128, B]; slot s at [s%128, s//128]; bf16 exact for ints <= 255
    dltile = dlvals.reshape(-1, P).T.astype(ml_dtypes.bfloat16).copy()
    return wrapped, dltile


def build_host_data(x, edge_index, W1, b1, W2, b2, ncores=8, sg_size=7):
    N, IN = x.shape
    H = W1.shape[1]
    OUT = W2.shape[1]
    assert N % ncores == 0
    shard = N // ncores
    ngroups = math.ceil(shard / P)
    shard_pad = ngroups * P
    table_rows = shard_pad * ncores
    assert table_rows % NCHUNK == 0
    chunk = table_rows // NCHUNK
    assert chunk - 1 < 2**15, "chunk too large for int16 gather idx"

    dims = dict(
        N=N,
        IN=IN,
        H=H,
        OUT=OUT,
        ncores=ncores,
        shard=shard,
        ngroups=ngroups,
        shard_pad=shard_pad,
        table_rows=table_rows,
        chunk=chunk,
        sg_size=sg_size,
    )

    src = np.concatenate([np.asarray(edge_index[0]), np.arange(N)]).astype(np.int64)
    dst = np.concatenate([np.asarray(edge_index[1]), np.arange(N)]).astype(np.int64)
    deg = np.bincount(dst, minlength=N)
    dis = 1.0 / np.sqrt(np.maximum(deg, 1.0))

    core = dst // shard
    dstloc = dst % shard
    eg = dstloc // P
    edl = (dstloc % P).astype(np.float32)

    # x table: rows in *padded shard* coordinates so that layer1 and layer2
    # tables share the same row mapping (row = shard_pad*(n//shard) + n%shard).
    trow = (src // shard) * shard_pad + (src % shard)
    xt = np.zeros((table_rows, IN), ml_dtypes.bfloat16)
    xs = np.asarray(x, np.float32) * dis[:, None]
    xrow = (np.arange(N) // shard) * shard_pad + (np.arange(N) % shard)
    xt[xrow] = xs.astype(ml_dtypes.bfloat16)

    ec = trow // chunk
    eloc = trow % chunk

    # both layers share the same (g, chunk) structure since table row mapping
    # is identical -> one schedule reused for both layers
    seg_len = np.zeros((ncores, ngroups, NCHUNK), np.int64)
    np.add.at(seg_len, (core, eg, ec), 1)
    schedule = make_schedule(dims, seg_len.max(axis=0))

    per_core = []
    for k in range(ncores):
        m = core == k
        wrapped, dltile = fill_core_slots(
            schedule, (eg[m], ec[m], eloc[m], edl[m]), dims
        )
        disn = np.zeros(shard_pad, np.float32)
        disn[:shard] = dis[k * shard : (k + 1) * shard]
        dis_t = disn.reshape(ngroups, P).T.copy()  # [128, ngroups]
        per_core.append(dict(idx=wrapped, dl=dltile, dis=dis_t))

    consts = dict(
        xt=xt,
        W1=np.asarray(W1, np.float32),
        W2=np.asarray(W2, np.float32),
        b1m=np.tile(np.asarray(b1, np.float32), (P, 1)),
        b2m=np.tile(np.asarray(b2, np.float32), (P, 1)),
        iota=np.tile(np.arange(P, dtype=ml_dtypes.bfloat16), (P, 1)),
    )
    return dims, schedule, consts, per_core


# -------------------------------------------------------------- device side


def build_kernel(nc, dims, schedule, variant="full"):
    dt = mybir.dt
    IN, H, OUT = dims["IN"], dims["H"], dims["OUT"]
    ncores = dims["ncores"]
    table_rows, chunk = dims["table_rows"], dims["chunk"]
    shard_pad = dims["shard_pad"]

    xt = nc.dram_tensor("xt", [table_rows, IN], dt.bfloat16, kind="ExternalInput")
    idx_in = nc.dram_tensor(
        "idx", [P, schedule["total_slots"] // 16], dt.int16, kind="ExternalInput"
    )
    dl_in = nc.dram_tensor(
        "dl", [P, schedule["total_batches"]], dt.bfloat16, kind="ExternalInput"
    )
    dis_in = nc.dram_tensor("dis", [P, dims["ngroups"]], dt.float32, kind="ExternalInput")
    W1_in = nc.dram_tensor("W1", [IN, H], dt.float32, kind="ExternalInput")
    W2_in = nc.dram_tensor("W2", [H, OUT], dt.float32, kind="ExternalInput")
    b1_in = nc.dram_tensor("b1m", [P, H], dt.float32, kind="ExternalInput")
    b2_in = nc.dram_tensor("b2m", [P, OUT], dt.float32, kind="ExternalInput")
    iota_in = nc.dram_tensor("iota", [P, P], dt.bfloat16, kind="ExternalInput")

    h1self = nc.dram_tensor("h1self", [shard_pad, H], dt.bfloat16, kind="Internal")
    h1full = nc.dram_tensor(
        "h1full",
        [table_rows, H],
        dt.bfloat16,
        kind="Internal",
        addr_space="Shared" if ncores > 4 else "Local",
    )
    out = nc.dram_tensor("out", [shard_pad, OUT], dt.float32, kind="ExternalOutput")

    maxb = schedule["max_sg_batches"]

    from concourse.library_config import mlp as mlp_lib

    with tile.TileContext(nc) as tc:
        nc.gpsimd.load_library(mlp_lib)

        # One shared Pool register per distinct gather length (48-reg budget).
        regcache = {}

        def nidx_reg(v):
            if v not in regcache:
                r = nc.gpsimd.alloc_register(f"nidx{v}")
                nc.gpsimd.reg_mov(r, v)
                regcache[v] = r
            return regcache[v]
        with (
            tc.tile_pool(name="const", bufs=1) as cpool,
            tc.tile_pool(name="gather", bufs=2) as gpool,
            tc.tile_pool(name="meta", bufs=2) as mpool,
            tc.tile_pool(name="oh", bufs=2) as ohpool,
            tc.tile_pool(name="ep", bufs=3) as epool,
            tc.tile_pool(name="aggp", bufs=2, space="PSUM") as aggpool,
            tc.tile_pool(name="densep", bufs=2, space="PSUM") as dpool,
        ):
            W1s = cpool.tile([IN, H], dt.float32)
            W2s = cpool.tile([H, OUT], dt.float32)
            b1s = cpool.tile([P, H], dt.float32)
            b2s = cpool.tile([P, OUT], dt.float32)
            iotas = cpool.tile([P, P], dt.bfloat16)
            diss = cpool.tile([P, dims["ngroups"]], dt.float32)
            nc.sync.dma_start(out=W1s[:], in_=W1_in[:, :])
            nc.sync.dma_start(out=W2s[:], in_=W2_in[:, :])
            nc.sync.dma_start(out=b1s[:], in_=b1_in[:, :])
            nc.sync.dma_start(out=b2s[:], in_=b2_in[:, :])
            nc.sync.dma_start(out=iotas[:], in_=iota_in[:, :])
            nc.sync.dma_start(out=diss[:], in_=dis_in[:, :])

            c01 = cpool.tile([P, 1], dt.float32)
            nc.vector.memset(c01[:], 0.1)

            # Tile assigns SWDGE completion sems round-robin over 8 lanes
            # (one per Pool-DMA instruction, in program order) and each sem
            # is locked to a single SWDGE queue.  Derive queue from the same
            # mod-8 counter so lane L always sees one queue.  Each queue is
            # served by its own Q7 core pair, so gather pieces generate
            # descriptors concurrently.  Queue 0 is left to the runtime
            # (mainline SWDGE is pinned there).
            # Each piece is <= SUBMAX idxs so a single_packet stream stays
            # at <= 64 descriptors per DMA engine (the packet spec ceiling);
            # single-desc packets (single_packet=False) drain ~3x slower.
            qmap = [1, 2, 3, 1, 2, 3, 1, 2]
            qi = 0
            SUBMAX = 1024

            layers = (0,) if variant == "layer1" else (0, 1)
            for layer in layers:
                table = xt if layer == 0 else h1full
                HH = H if layer == 0 else OUT
                Wt = W1s if layer == 0 else W2s
                bt = b1s if layer == 0 else b2s

                for s in schedule["sgs"]:
                    nb = s["nbatches"]
                    gtile = gpool.tile([P, maxb * P], dt.bfloat16, tag="g")
                    itile = mpool.tile(
                        [P, schedule["max_sg_batches"] * 8], dt.int16, tag="i"
                    )
                    dtile = mpool.tile([P, maxb], dt.bfloat16, tag="d")
                    nc.sync.dma_start(
                        out=itile[:, : s["idx_ncol"]],
                        in_=idx_in[:, s["idx_col"] : s["idx_col"] + s["idx_ncol"]],
                    )
                    nc.sync.dma_start(
                        out=dtile[:, :nb],
                        in_=dl_in[:, s["batch_off"] : s["batch_off"] + nb],
                    )
                    for cnum, clen, coff, boff in s["calls"]:
                        for off in range(0, clen, SUBMAX):
                            plen = min(SUBMAX, clen - off)
                            o0 = boff * P + off
                            c0 = coff - s["idx_col"] + off // 16
                            nc.gpsimd.dma_gather(
                                out_ap=gtile[:, o0 : o0 + plen].rearrange(
                                    "p (b f) -> p b f", f=P
                                ),
                                in_ap=table[cnum * chunk : (cnum + 1) * chunk, :],
                                idxs_ap=itile[:, c0 : c0 + plen // 16],
                                num_idxs=plen,
                                num_idxs_reg=nidx_reg(plen),
                                elem_size=IN if layer == 0 else H,
                                single_packet=True,
                                queue_num=qmap[qi % 8],
                            )
                            qi += 1
                    # One-hot matrix for every batch of this sg in one DVE op:
                    # oh[p, b, j] = (dl[p, b] == j).  tensor_tensor runs in
                    # single-port mode so it never locks GpSimd out of the
                    # shared SBUF port pair (tensor_scalar would).
                    ohtile = ohpool.tile([P, maxb * P], dt.bfloat16, tag="oh")
                    nc.vector.tensor_tensor(
                        out=ohtile[:, : nb * P].rearrange("p (b f) -> p b f", f=P),
                        in0=dtile[:, :nb].unsqueeze(2).broadcast_to([P, nb, P]),
                        in1=iotas[:, :].unsqueeze(1).broadcast_to([P, nb, P]),
                        op=mybir.AluOpType.is_equal,
                    )
                    for gg, bl in s["groups"]:
                        agg = aggpool.tile([P, P], dt.float32, tag="agg")
                        for j, b in enumerate(bl):
                            nc.tensor.matmul(
                                out=agg[:],
                                lhsT=gtile[:, b * P : (b + 1) * P],
                                rhs=ohtile[:, b * P : (b + 1) * P],
                                start=(j == 0),
                                stop=(j == len(bl) - 1),
                            )
                        aggs = epool.tile([P, P], dt.float32, tag="aggs")
                        nc.vector.tensor_copy(out=aggs[:], in_=agg[:])
                        hraw = dpool.tile([P, HH], dt.float32, tag="hraw")
                        nc.tensor.matmul(
                            out=hraw[:], lhsT=aggs[:], rhs=Wt[:], start=True, stop=True
                        )
                        t1 = epool.tile([P, HH], dt.float32, tag="t1")
                        nc.vector.scalar_tensor_tensor(
                            out=t1[:],
                            in0=hraw[:],
                            scalar=diss[:, gg : gg + 1],
                            in1=bt[:],
                            op0=mybir.AluOpType.mult,
                            op1=mybir.AluOpType.add,
                        )
                        if layer == 0:
                            t2 = epool.tile([P, HH], dt.float32, tag="t2")
                            nc.scalar.activation(
                                out=t2[:], in_=t1[:], func=mybir.ActivationFunctionType.Relu
                            )
                            hst = epool.tile([P, HH], dt.bfloat16, tag="hst")
                            nc.vector.tensor_tensor(
                                out=hst[:],
                                in0=t2[:],
                                in1=diss[:, gg : gg + 1].broadcast_to([P, HH]),
                                op=mybir.AluOpType.mult,
                            )
                            nc.sync.dma_start(
                                out=h1self[gg * P : (gg + 1) * P, :], in_=hst[:]
                            )
                        else:
                            t2 = epool.tile([P, HH], dt.float32, tag="t2")
                            nc.scalar.activation(
                                out=t2[:],
                                in_=t1[:],
                                func=mybir.ActivationFunctionType.Sigmoid,
                            )
                            ot = epool.tile([P, HH], dt.float32, tag="ot")
                            nc.vector.scalar_tensor_tensor(
                                out=ot[:],
                                in0=t2[:],
                                scalar=0.8,
                                in1=c01[:, 0:1].broadcast_to([P, HH]),
                                op0=mybir.AluOpType.mult,
                                op1=mybir.AluOpType.add,
                            )
                            nc.sync.dma_start(
                                out=out[gg * P : (gg + 1) * P, :], in_=ot[:]
                            )
                if layer == 0 and variant == "full":
                    nc.gpsimd.collective_compute(
                        kind="AllGather",
                        op=mybir.AluOpType.bypass,
                        replica_groups=[list(range(ncores))],
                        ins=[h1self[:, :]],
                        outs=[h1full[:, :]],
                    )
                elif layer == 0 and variant == "nocoll":
                    nc.sync.dma_start(out=h1full[:shard_pad, :], in_=h1self[:, :])
    return nc


def make_in_maps(dims, consts, per_core):
    in_maps = []
    for pc in per_core:
        in_maps.append(
            dict(
                xt=consts["xt"],
                idx=pc["idx"],
                dl=pc["dl"],
                dis=pc["dis"],
                W1=consts["W1"],
                W2=consts["W2"],
                b1m=consts["b1m"],
                b2m=consts["b2m"],
                iota=consts["iota"],
            )
        )
    return in_maps


def _install_ntff_hook():
    """Provide antenv.axon_hooks (missing on this image) so that
    run_bass_kernel_spmd(trace=True) can capture NTFF profiles via the
    axon .so's NRT-profile C ABI."""
    import sys
    import types

    if "antenv.axon_hooks" in sys.modules:
        return
    try:
        import antenv
        from trn_agent_boot.trn_boot import _ntff_profile_via_ctypes

        hook = _ntff_profile_via_ctypes("/opt/axon/libaxon_pjrt.so")
        mod = types.ModuleType("antenv.axon_hooks")
        mod._hook = hook

        def get_axon_ntff_profile_hook():
            return mod._hook

        def set_axon_ntff_profile_hook(h):
            mod._hook = h

        mod.get_axon_ntff_profile_hook = get_axon_ntff_profile_hook
        mod.set_axon_ntff_profile_hook = set_axon_ntff_profile_hook
        sys.modules["antenv.axon_hooks"] = mod
        antenv.axon_hooks = mod
    except Exception as e:  # pragma: no cover
        print("ntff hook install failed:", e)


def run(x, edge_index, W1, b1, W2, b2, ncores=8, sg_size=7, trace=False, variant="full"):
    from concourse import bass_utils

    if trace:
        _install_ntff_hook()

    dims, schedule, consts, per_core = build_host_data(
        x, edge_index, W1, b1, W2, b2, ncores=ncores, sg_size=sg_size
    )
    nc = bacc.Bacc(num_devices=ncores, num_swdge_queues=4)
    build_kernel(nc, dims, schedule, variant=variant)
    nc.compile()
    in_maps = make_in_maps(dims, consts, per_core)
    res = bass_utils.run_bass_kernel_spmd(
        nc, in_maps, core_ids=list(range(ncores)), trace=trace
    )
    shard, shard_pad = dims["shard"], dims["shard_pad"]
    full = np.concatenate([r["out"][:shard] for r in res.results], axis=0)
    return full, res


# ------------------------------------------------------------- harness entry


def kernel(**inputs):
    """Full (unsharded) inputs -> full output, computed on 8 NeuronCores."""
    out, _ = run(
        np.asarray(inputs["x"], np.float32),
        np.asarray(inputs["edge_index"]),
        np.asarray(inputs["W1"], np.float32),
        np.asarray(inputs["b1"], np.float32),
        np.asarray(inputs["W2"], np.float32),
        np.asarray(inputs["b2"], np.float32),
        ncores=8,
        sg_size=7,
        trace=False,
    )
    return out.astype(np.float32)



# revision 26
# speedup vs baseline: 2.8028x; 1.5517x over previous
"""2-layer GCN (GCNConv -> relu -> GCNConv -> sigmoid affine) on TRN2, SPMD over NCORES.

Strategy:
  - Nodes (dst) sharded across cores; edges partitioned by dst shard.
  - Per core, edges sorted into dst-groups of 128, then by src table chunk
    (dma_gather idx is int16 -> gather tables are split into 4 chunks).
  - Aggregation:  aggT[feat, dst128] += msg[e, feat].T @ onehot[e, dst128]
    where msg rows are dma_gather'ed (bf16, dis-prescaled tables) and the
    onehot is built with one DVE tensor_scalar is_equal against an iota row.
  - GCN linearity:  A_hat (x W) == (A_hat x) W, so the dense W matmul runs
    once per 128-dst group on the aggregated tile (fp32).
  - Layer1 output (dis-prescaled, bf16) is AllGather'ed into a full table
    which layer2 gathers from.
"""

import math

import numpy as np
import ml_dtypes

import concourse.bass as bass
import concourse.mybir as mybir
import concourse.tile as tile
from concourse import bacc

P = 128
NCHUNK = 4


# ---------------------------------------------------------------- host side


def make_schedule(dims, seg_len_max):
    """Static (core-independent) schedule.

    seg_len_max: [ngroups, NCHUNK] max-over-cores segment length (edges with
    dst in group g whose table row falls in chunk c).

    Returns dict with per-supergroup call/batch layout.
    """
    ngroups, sg_size = dims["ngroups"], dims["sg_size"]
    pad_len = (np.ceil(seg_len_max / P).astype(np.int64)) * P  # [ngroups, NCHUNK]
    nsg = math.ceil(ngroups / sg_size)
    sgs = []
    slot_off = 0  # slots, across whole layer
    idx_off = 0  # int16 idx columns (16 rows) across whole layer
    batch_off = 0
    # Quantize call lengths so there are few distinct num_idxs values: each
    # distinct value costs one Pool register (48 total on the engine).
    lens = []
    for s in range(nsg):
        groups = list(range(s * sg_size, min((s + 1) * sg_size, ngroups)))
        for c in range(NCHUNK):
            lens.append(int(sum(pad_len[g, c] for g in groups)))
    quant = P
    while len({-(-l // quant) * quant for l in lens if l > 0}) > 16:
        quant *= 2

    for s in range(nsg):
        groups = list(range(s * sg_size, min((s + 1) * sg_size, ngroups)))
        calls = []  # (chunk, num_idxs, idx_col_off_abs, batch_off_in_sg)
        seg_slot = {}  # (g, c) -> slot offset within sg
        sg_slots = 0
        for c in range(NCHUNK):
            call_len = int(sum(pad_len[g, c] for g in groups))
            call_pad = -(-call_len // quant) * quant
            if call_pad > 0:
                calls.append((c, call_pad, idx_off + sg_slots // 16, sg_slots // P))
            for g in groups:
                seg_slot[(g, c)] = sg_slots
                sg_slots += int(pad_len[g, c])
            sg_slots += call_pad - call_len
        gbatches = []  # (g, [batch indices within sg])
        for g in groups:
            bl = []
            for c in range(NCHUNK):
                base = seg_slot[(g, c)] // P
                bl.extend(range(base, base + int(pad_len[g, c]) // P))
            gbatches.append((g, bl))
        sgs.append(
            dict(
                calls=calls,
                groups=gbatches,
                nbatches=sg_slots // P,
                idx_col=idx_off,  # absolute idx col offset of this sg
                idx_ncol=sg_slots // 16,
                batch_off=batch_off,
                slot_off=slot_off,
            )
        )
        slot_off += sg_slots
        idx_off += sg_slots // 16
        batch_off += sg_slots // P
    return dict(
        sgs=sgs,
        total_slots=slot_off,
        total_batches=batch_off,
        max_sg_batches=max(s["nbatches"] for s in sgs),
        pad_len=pad_len,
    )


def fill_core_slots(schedule, core_edges, dims):
    """Build per-core idx (int16 wrapped [16, T/16]) and dl (bf16 [128, B]) arrays.

    core_edges: (g, c, loc, dl) int arrays for this core's edges, any order.
    """
    ngroups = dims["ngroups"]
    g, c, loc, dl = core_edges
    total_slots = schedule["total_slots"]
    idxvals = np.zeros(total_slots, np.int16)
    dlvals = np.full(total_slots, 255.0, np.float32)  # 255 -> all-zero onehot col

    # segment base slots (absolute): recompute from schedule
    seg_base = np.zeros((ngroups, NCHUNK), np.int64)
    for s in schedule["sgs"]:
        off = s["slot_off"]
        pads = schedule["pad_len"]
        for cc in range(NCHUNK):
            for gg, _bl in s["groups"]:
                seg_base[gg, cc] = off
                off += int(pads[gg, cc])

    key = g * NCHUNK + c
    order = np.argsort(key, kind="stable")
    key_s = key[order]
    # rank within segment
    seg_start = np.searchsorted(key_s, np.arange(ngroups * NCHUNK))
    rank = np.arange(len(key_s)) - seg_start[key_s]
    pos = seg_base[g[order], c[order]] + rank
    idxvals[pos] = loc[order].astype(np.int16)
    dlvals[pos] = dl[order]

    wrapped = idxvals.reshape(-1, 16).T  # [16, T/16]; idx i at [i%16, i//16]
    wrapped = np.tile(wrapped, (8, 1)).copy()  # replicated for the 8 Q7 cores
    # [128, B]; slot s at [s%128, s//128]; bf16 exact for ints <= 255
    dltile = dlvals.reshape(-1, P).T.astype(ml_dtypes.bfloat16).copy()
    return wrapped, dltile


def build_host_data(x, edge_index, W1, b1, W2, b2, ncores=8, sg_size=7):
    N, IN = x.shape
    H = W1.shape[1]
    OUT = W2.shape[1]
    ngroups_abs = math.ceil(N / P)
    ngroups = math.ceil(ngroups_abs / ncores)
    shard_pad = ngroups * P
    table_rows = shard_pad * ncores
    assert table_rows % NCHUNK == 0
    chunk = table_rows // NCHUNK
    assert chunk - 1 < 2**15, "chunk too large for int16 gather idx"

    dims = dict(
        N=N,
        IN=IN,
        H=H,
        OUT=OUT,
        ncores=ncores,
        ngroups=ngroups,
        shard_pad=shard_pad,
        table_rows=table_rows,
        chunk=chunk,
        sg_size=sg_size,
    )

    # Degrees include the self-loops, but the self-loop term itself is added
    # on-device via one identity matmul per dst group (agg[f,d] += own[d,f]
    # is a PE transpose) — gathering self rows would concentrate 128 edges of
    # one core into one (slot, chunk) segment and pad every other core.
    dst_e = np.asarray(edge_index[1])
    deg = np.bincount(
        np.concatenate([dst_e, np.arange(N)]), minlength=N
    )
    dis = 1.0 / np.sqrt(np.maximum(deg, 1.0))
    src = np.asarray(edge_index[0]).astype(np.int64)
    dst = dst_e.astype(np.int64)

    # Balanced group->-(core, slot) assignment: per-(slot, chunk) gather
    # segments are padded to the max over cores, so matching similar-sized
    # dst-groups in the same slot minimizes that padding.  Sort the 128-node
    # groups by edge count; rank r -> core r%ncores, slot r//ncores.
    gcount = np.bincount(dst // P, minlength=ngroups_abs)
    rank = np.empty(ngroups_abs, np.int64)
    rank[np.argsort(-gcount, kind="stable")] = np.arange(ngroups_abs)
    core_of = rank % ncores
    slot_of = rank // ncores

    gd = dst // P
    core = core_of[gd]
    eg = slot_of[gd]
    edl = (dst % P).astype(np.float32)

    # node n -> table row (same mapping for the x table and the h1 table)
    n_all = np.arange(N)
    gn = n_all // P
    noderow = core_of[gn] * shard_pad + slot_of[gn] * P + (n_all % P)
    trow = noderow[src]
    xt = np.zeros((table_rows, IN), ml_dtypes.bfloat16)
    xs = np.asarray(x, np.float32) * dis[:, None]
    xt[noderow] = xs.astype(ml_dtypes.bfloat16)

    ec = trow // chunk
    eloc = trow % chunk

    # both layers share the same (g, chunk) structure since table row mapping
    # is identical -> one schedule reused for both layers
    seg_len = np.zeros((ncores, ngroups, NCHUNK), np.int64)
    np.add.at(seg_len, (core, eg, ec), 1)
    schedule = make_schedule(dims, seg_len.max(axis=0))

    per_core = []
    for k in range(ncores):
        m = core == k
        wrapped, dltile = fill_core_slots(
            schedule, (eg[m], ec[m], eloc[m], edl[m]), dims
        )
        disn = np.zeros(shard_pad, np.float32)
        nm = core_of[gn] == k
        disn[slot_of[gn[nm]] * P + (n_all[nm] % P)] = dis[nm]
        dis_t = disn.reshape(ngroups, P).T.copy()  # [128, ngroups]
        per_core.append(
            dict(
                idx=wrapped,
                dl=dltile,
                dis=dis_t,
                xself=xt[k * shard_pad : (k + 1) * shard_pad].copy(),
            )
        )

    dims["out_core"] = core_of[gn]
    dims["out_row"] = slot_of[gn] * P + (n_all % P)

    consts = dict(
        xt=xt,
        W1=np.asarray(W1, np.float32),
        W2=np.asarray(W2, np.float32),
        b1m=np.tile(np.asarray(b1, np.float32), (P, 1)),
        b2m=np.tile(np.asarray(b2, np.float32), (P, 1)),
        iota=np.tile(np.arange(P, dtype=ml_dtypes.bfloat16), (P, 1)),
        ident=np.eye(P, dtype=ml_dtypes.bfloat16),
    )
    return dims, schedule, consts, per_core


# -------------------------------------------------------------- device side


def build_kernel(nc, dims, schedule, variant="full"):
    dt = mybir.dt
    IN, H, OUT = dims["IN"], dims["H"], dims["OUT"]
    ncores = dims["ncores"]
    table_rows, chunk = dims["table_rows"], dims["chunk"]
    shard_pad = dims["shard_pad"]

    xt = nc.dram_tensor("xt", [table_rows, IN], dt.bfloat16, kind="ExternalInput")
    idx_in = nc.dram_tensor(
        "idx", [P, schedule["total_slots"] // 16], dt.int16, kind="ExternalInput"
    )
    dl_in = nc.dram_tensor(
        "dl", [P, schedule["total_batches"]], dt.bfloat16, kind="ExternalInput"
    )
    dis_in = nc.dram_tensor("dis", [P, dims["ngroups"]], dt.float32, kind="ExternalInput")
    W1_in = nc.dram_tensor("W1", [IN, H], dt.float32, kind="ExternalInput")
    W2_in = nc.dram_tensor("W2", [H, OUT], dt.float32, kind="ExternalInput")
    b1_in = nc.dram_tensor("b1m", [P, H], dt.float32, kind="ExternalInput")
    b2_in = nc.dram_tensor("b2m", [P, OUT], dt.float32, kind="ExternalInput")
    iota_in = nc.dram_tensor("iota", [P, P], dt.bfloat16, kind="ExternalInput")
    ident_in = nc.dram_tensor("ident", [P, P], dt.bfloat16, kind="ExternalInput")
    xself_in = nc.dram_tensor(
        "xself", [shard_pad, IN], dt.bfloat16, kind="ExternalInput"
    )

    h1self = nc.dram_tensor("h1self", [shard_pad, H], dt.bfloat16, kind="Internal")
    h1full = nc.dram_tensor(
        "h1full",
        [table_rows, H],
        dt.bfloat16,
        kind="Internal",
        addr_space="Shared" if ncores > 4 else "Local",
    )
    out = nc.dram_tensor("out", [shard_pad, OUT], dt.float32, kind="ExternalOutput")

    maxb = schedule["max_sg_batches"]

    from concourse.library_config import mlp as mlp_lib

    with tile.TileContext(nc) as tc:
        nc.gpsimd.load_library(mlp_lib)

        # One shared Pool register per distinct gather length (48-reg budget).
        regcache = {}

        def nidx_reg(v):
            if v not in regcache:
                r = nc.gpsimd.alloc_register(f"nidx{v}")
                nc.gpsimd.reg_mov(r, v)
                regcache[v] = r
            return regcache[v]
        with (
            tc.tile_pool(name="const", bufs=1) as cpool,
            tc.tile_pool(name="gather", bufs=2) as gpool,
            tc.tile_pool(name="meta", bufs=2) as mpool,
            tc.tile_pool(name="oh", bufs=2) as ohpool,
            tc.tile_pool(name="own", bufs=3) as opool,
            tc.tile_pool(name="ep", bufs=3) as epool,
            tc.tile_pool(name="aggp", bufs=2, space="PSUM") as aggpool,
            tc.tile_pool(name="densep", bufs=2, space="PSUM") as dpool,
        ):
            W1s = cpool.tile([IN, H], dt.float32)
            W2s = cpool.tile([H, OUT], dt.float32)
            b1s = cpool.tile([P, H], dt.float32)
            b2s = cpool.tile([P, OUT], dt.float32)
            iotas = cpool.tile([P, P], dt.bfloat16)
            idents = cpool.tile([P, P], dt.bfloat16)
            diss = cpool.tile([P, dims["ngroups"]], dt.float32)
            nc.sync.dma_start(out=W1s[:], in_=W1_in[:, :])
            nc.sync.dma_start(out=W2s[:], in_=W2_in[:, :])
            nc.sync.dma_start(out=b1s[:], in_=b1_in[:, :])
            nc.sync.dma_start(out=b2s[:], in_=b2_in[:, :])
            nc.sync.dma_start(out=iotas[:], in_=iota_in[:, :])
            nc.sync.dma_start(out=idents[:], in_=ident_in[:, :])
            nc.sync.dma_start(out=diss[:], in_=dis_in[:, :])

            c01 = cpool.tile([P, 1], dt.float32)
            nc.vector.memset(c01[:], 0.1)

            # Tile assigns SWDGE completion sems round-robin over 8 lanes
            # (one per Pool-DMA instruction, in program order) and each sem
            # is locked to a single SWDGE queue.  Derive queue from the same
            # mod-8 counter so lane L always sees one queue.  Each queue is
            # served by its own Q7 core pair, so gather pieces generate
            # descriptors concurrently on all four pairs.
            # Each piece is <= SUBMAX idxs so a single_packet stream stays
            # at <= 64 descriptors per DMA engine (the packet spec ceiling);
            # single-desc packets (single_packet=False) drain ~3x slower.
            qmap = [0, 1, 2, 3, 0, 1, 2, 3]
            qi = 0
            SUBMAX = 1024

            layers = (0,) if variant == "layer1" else (0, 1)
            for layer in layers:
                table = xt if layer == 0 else h1full
                HH = H if layer == 0 else OUT
                Wt = W1s if layer == 0 else W2s
                bt = b1s if layer == 0 else b2s

                for s in schedule["sgs"]:
                    nb = s["nbatches"]
                    gtile = gpool.tile([P, maxb * P], dt.bfloat16, tag="g")
                    itile = mpool.tile(
                        [P, schedule["max_sg_batches"] * 8], dt.int16, tag="i"
                    )
                    dtile = mpool.tile([P, maxb], dt.bfloat16, tag="d")
                    nc.sync.dma_start(
                        out=itile[:, : s["idx_ncol"]],
                        in_=idx_in[:, s["idx_col"] : s["idx_col"] + s["idx_ncol"]],
                    )
                    nc.sync.dma_start(
                        out=dtile[:, :nb],
                        in_=dl_in[:, s["batch_off"] : s["batch_off"] + nb],
                    )
                    # Issue pieces round-robin ACROSS chunk calls: the first
                    # groups' batches sit at the start of every chunk's
                    # segment, so interleaving lands their data first and the
                    # matmuls overlap the remaining descriptor generation.
                    pieces = []
                    for cnum, clen, coff, boff in s["calls"]:
                        for off in range(0, clen, SUBMAX):
                            pieces.append((off // SUBMAX, cnum, clen, coff, boff, off))
                    pieces.sort()
                    for _, cnum, clen, coff, boff, off in pieces:
                        plen = min(SUBMAX, clen - off)
                        o0 = boff * P + off
                        c0 = coff - s["idx_col"] + off // 16
                        nc.gpsimd.dma_gather(
                            out_ap=gtile[:, o0 : o0 + plen].rearrange(
                                "p (b f) -> p b f", f=P
                            ),
                            in_ap=table[cnum * chunk : (cnum + 1) * chunk, :],
                            idxs_ap=itile[:, c0 : c0 + plen // 16],
                            num_idxs=plen,
                            num_idxs_reg=nidx_reg(plen),
                            elem_size=IN if layer == 0 else H,
                            single_packet=True,
                            queue_num=qmap[qi % 8],
                        )
                        qi += 1
                    # One-hot matrix for every batch of this sg in one DVE op:
                    # oh[p, b, j] = (dl[p, b] == j).  tensor_tensor runs in
                    # single-port mode so it never locks GpSimd out of the
                    # shared SBUF port pair (tensor_scalar would).
                    ohtile = ohpool.tile([P, maxb * P], dt.bfloat16, tag="oh")
                    nc.vector.tensor_tensor(
                        out=ohtile[:, : nb * P].rearrange("p (b f) -> p b f", f=P),
                        in0=dtile[:, :nb].unsqueeze(2).broadcast_to([P, nb, P]),
                        in1=iotas[:, :].unsqueeze(1).broadcast_to([P, nb, P]),
                        op=mybir.AluOpType.is_equal,
                    )
                    # Self-loop term: agg[f, d] += own[d, f] — a PE transpose
                    # via an identity matmul in the group's PSUM chain.
                    for gg, bl in s["groups"]:
                        own = opool.tile([P, H], dt.bfloat16, tag="own")
                        ownsrc = xself_in if layer == 0 else h1self
                        nc.sync.dma_start(
                            out=own[:], in_=ownsrc[gg * P : (gg + 1) * P, :]
                        )
                        agg = aggpool.tile([P, P], dt.float32, tag="agg")
                        nc.tensor.matmul(
                            out=agg[:],
                            lhsT=own[:],
                            rhs=idents[:],
                            start=True,
                            stop=(len(bl) == 0),
                        )
                        for j, b in enumerate(bl):
                            nc.tensor.matmul(
                                out=agg[:],
                                lhsT=gtile[:, b * P : (b + 1) * P],
                                rhs=ohtile[:, b * P : (b + 1) * P],
                                start=False,
                                stop=(j == len(bl) - 1),
                            )
                        aggs = epool.tile([P, P], dt.float32, tag="aggs")
                        nc.vector.tensor_copy(out=aggs[:], in_=agg[:])
                        hraw = dpool.tile([P, HH], dt.float32, tag="hraw")
                        nc.tensor.matmul(
                            out=hraw[:], lhsT=aggs[:], rhs=Wt[:], start=True, stop=True
                        )
                        t1 = epool.tile([P, HH], dt.float32, tag="t1")
                        nc.vector.scalar_tensor_tensor(
                            out=t1[:],
                            in0=hraw[:],
                            scalar=diss[:, gg : gg + 1],
                            in1=bt[:],
                            op0=mybir.AluOpType.mult,
                            op1=mybir.AluOpType.add,
                        )
                        if layer == 0:
                            t2 = epool.tile([P, HH], dt.float32, tag="t2")
                            nc.scalar.activation(
                                out=t2[:], in_=t1[:], func=mybir.ActivationFunctionType.Relu
                            )
                            hst = epool.tile([P, HH], dt.bfloat16, tag="hst")
                            nc.vector.tensor_tensor(
                                out=hst[:],
                                in0=t2[:],
                                in1=diss[:, gg : gg + 1].broadcast_to([P, HH]),
                                op=mybir.AluOpType.mult,
                            )
                            nc.sync.dma_start(
                                out=h1self[gg * P : (gg + 1) * P, :], in_=hst[:]
                            )
                        else:
                            t2 = epool.tile([P, HH], dt.float32, tag="t2")
                            nc.scalar.activation(
                                out=t2[:],
                                in_=t1[:],
                                func=mybir.ActivationFunctionType.Sigmoid,
                            )
                            ot = epool.tile([P, HH], dt.float32, tag="ot")
                            nc.vector.scalar_tensor_tensor(
                                out=ot[:],
                                in0=t2[:],
                                scalar=0.8,
                                in1=c01[:, 0:1].broadcast_to([P, HH]),
                                op0=mybir.AluOpType.mult,
                                op1=mybir.AluOpType.add,
                            )
                            nc.sync.dma_start(
                                out=out[gg * P : (gg + 1) * P, :], in_=ot[:]
                            )
                if layer == 0 and variant == "full":
                    nc.gpsimd.collective_compute(
                        kind="AllGather",
                        op=mybir.AluOpType.bypass,
                        replica_groups=[list(range(ncores))],
                        ins=[h1self[:, :]],
                        outs=[h1full[:, :]],
                    )
                elif layer == 0 and variant == "nocoll":
                    nc.sync.dma_start(out=h1full[:shard_pad, :], in_=h1self[:, :])
    return nc


def make_in_maps(dims, consts, per_core):
    in_maps = []
    for pc in per_core:
        in_maps.append(
            dict(
                xt=consts["xt"],
                idx=pc["idx"],
                dl=pc["dl"],
                dis=pc["dis"],
                xself=pc["xself"],
                W1=consts["W1"],
                W2=consts["W2"],
                b1m=consts["b1m"],
                b2m=consts["b2m"],
                iota=consts["iota"],
                ident=consts["ident"],
            )
        )
    return in_maps


def _install_ntff_hook():
    """Provide antenv.axon_hooks (missing on this image) so that
    run_bass_kernel_spmd(trace=True) can capture NTFF profiles via the
    axon .so's NRT-profile C ABI."""
    import sys
    import types

    if "antenv.axon_hooks" in sys.modules:
        return
    try:
        import antenv
        from trn_agent_boot.trn_boot import _ntff_profile_via_ctypes

        hook = _ntff_profile_via_ctypes("/opt/axon/libaxon_pjrt.so")
        mod = types.ModuleType("antenv.axon_hooks")
        mod._hook = hook

        def get_axon_ntff_profile_hook():
            return mod._hook

        def set_axon_ntff_profile_hook(h):
            mod._hook = h

        mod.get_axon_ntff_profile_hook = get_axon_ntff_profile_hook
        mod.set_axon_ntff_profile_hook = set_axon_ntff_profile_hook
        sys.modules["antenv.axon_hooks"] = mod
        antenv.axon_hooks = mod
    except Exception as e:  # pragma: no cover
        print("ntff hook install failed:", e)


def run(x, edge_index, W1, b1, W2, b2, ncores=8, sg_size=7, trace=False, variant="full"):
    from concourse import bass_utils

    if trace:
        _install_ntff_hook()

    dims, schedule, consts, per_core = build_host_data(
        x, edge_index, W1, b1, W2, b2, ncores=ncores, sg_size=sg_size
    )
    nc = bacc.Bacc(num_devices=ncores, num_swdge_queues=4)
    build_kernel(nc, dims, schedule, variant=variant)
    nc.compile()
    in_maps = make_in_maps(dims, consts, per_core)
    res = bass_utils.run_bass_kernel_spmd(
        nc, in_maps, core_ids=list(range(ncores)), trace=trace
    )
    allout = np.stack([r["out"] for r in res.results])  # [ncores, shard_pad, OUT]
    full = allout[dims["out_core"], dims["out_row"]]
    return full, res


# ------------------------------------------------------------- harness entry


def kernel(**inputs):
    """Full (unsharded) inputs -> full output, computed on 8 NeuronCores."""
    out, _ = run(
        np.asarray(inputs["x"], np.float32),
        np.asarray(inputs["edge_index"]),
        np.asarray(inputs["W1"], np.float32),
        np.asarray(inputs["b1"], np.float32),
        np.asarray(inputs["W2"], np.float32),
        np.asarray(inputs["b2"], np.float32),
        ncores=8,
        sg_size=7,
        trace=False,
    )
    return out.astype(np.float32)



# revision 33
# speedup vs baseline: 2.8694x; 1.0238x over previous
"""2-layer GCN (GCNConv -> relu -> GCNConv -> sigmoid affine) on TRN2, SPMD over NCORES.

Strategy:
  - Nodes (dst) sharded across cores; edges partitioned by dst shard.
  - Per core, edges sorted into dst-groups of 128, then by src table chunk
    (dma_gather idx is int16 -> gather tables are split into 4 chunks).
  - Aggregation:  aggT[feat, dst128] += msg[e, feat].T @ onehot[e, dst128]
    where msg rows are dma_gather'ed (bf16, dis-prescaled tables) and the
    onehot is built with one DVE tensor_scalar is_equal against an iota row.
  - GCN linearity:  A_hat (x W) == (A_hat x) W, so the dense W matmul runs
    once per 128-dst group on the aggregated tile (fp32).
  - Layer1 output (dis-prescaled, bf16) is AllGather'ed into a full table
    which layer2 gathers from.
"""

import math

import numpy as np
import ml_dtypes

import concourse.bass as bass
import concourse.mybir as mybir
import concourse.tile as tile
from concourse import bacc

P = 128
NCHUNK = 4


# ---------------------------------------------------------------- host side


def make_schedule(dims, seg_len_max):
    """Static (core-independent) schedule.

    seg_len_max: [ngroups, NCHUNK] max-over-cores segment length (edges with
    dst in group g whose table row falls in chunk c).

    Returns dict with per-supergroup call/batch layout.
    """
    ngroups, sg_size = dims["ngroups"], dims["sg_size"]
    pad_len = (np.ceil(seg_len_max / P).astype(np.int64)) * P  # [ngroups, NCHUNK]
    nsg = math.ceil(ngroups / sg_size)
    sgs = []
    slot_off = 0  # slots, across whole layer
    idx_off = 0  # int16 idx columns (16 rows) across whole layer
    batch_off = 0
    # Quantize call lengths so there are few distinct num_idxs values: each
    # distinct value costs one Pool register (48 total on the engine).
    lens = []
    for s in range(nsg):
        groups = list(range(s * sg_size, min((s + 1) * sg_size, ngroups)))
        for c in range(NCHUNK):
            lens.append(int(sum(pad_len[g, c] for g in groups)))
    quant = P
    while len({-(-l // quant) * quant for l in lens if l > 0}) > 16:
        quant *= 2

    for s in range(nsg):
        groups = list(range(s * sg_size, min((s + 1) * sg_size, ngroups)))
        calls = []  # (chunk, num_idxs, idx_col_off_abs, batch_off_in_sg)
        seg_slot = {}  # (g, c) -> slot offset within sg
        sg_slots = 0
        for c in range(NCHUNK):
            call_len = int(sum(pad_len[g, c] for g in groups))
            call_pad = -(-call_len // quant) * quant
            if call_pad > 0:
                calls.append((c, call_pad, idx_off + sg_slots // 16, sg_slots // P))
            for g in groups:
                seg_slot[(g, c)] = sg_slots
                sg_slots += int(pad_len[g, c])
            sg_slots += call_pad - call_len
        gbatches = []  # (g, [batch indices within sg])
        for g in groups:
            bl = []
            for c in range(NCHUNK):
                base = seg_slot[(g, c)] // P
                bl.extend(range(base, base + int(pad_len[g, c]) // P))
            gbatches.append((g, bl))
        sgs.append(
            dict(
                calls=calls,
                groups=gbatches,
                nbatches=sg_slots // P,
                idx_col=idx_off,  # absolute idx col offset of this sg
                idx_ncol=sg_slots // 16,
                batch_off=batch_off,
                slot_off=slot_off,
            )
        )
        slot_off += sg_slots
        idx_off += sg_slots // 16
        batch_off += sg_slots // P
    return dict(
        sgs=sgs,
        total_slots=slot_off,
        total_batches=batch_off,
        max_sg_batches=max(s["nbatches"] for s in sgs),
        pad_len=pad_len,
    )


def fill_core_slots(schedule, core_edges, dims):
    """Build per-core idx (int16 wrapped [16, T/16]) and dl (bf16 [128, B]) arrays.

    core_edges: (g, c, loc, dl) int arrays for this core's edges, any order.
    """
    ngroups = dims["ngroups"]
    g, c, loc, dl = core_edges
    total_slots = schedule["total_slots"]
    idxvals = np.zeros(total_slots, np.int16)
    dlvals = np.full(total_slots, 255.0, np.float32)  # 255 -> all-zero onehot col

    # segment base slots (absolute): recompute from schedule
    seg_base = np.zeros((ngroups, NCHUNK), np.int64)
    for s in schedule["sgs"]:
        off = s["slot_off"]
        pads = schedule["pad_len"]
        for cc in range(NCHUNK):
            for gg, _bl in s["groups"]:
                seg_base[gg, cc] = off
                off += int(pads[gg, cc])

    key = g * NCHUNK + c
    order = np.argsort(key, kind="stable")
    key_s = key[order]
    # rank within segment
    seg_start = np.searchsorted(key_s, np.arange(ngroups * NCHUNK))
    rank = np.arange(len(key_s)) - seg_start[key_s]
    pos = seg_base[g[order], c[order]] + rank
    idxvals[pos] = loc[order].astype(np.int16)
    dlvals[pos] = dl[order]

    wrapped = idxvals.reshape(-1, 16).T  # [16, T/16]; idx i at [i%16, i//16]
    wrapped = np.tile(wrapped, (8, 1)).copy()  # replicated for the 8 Q7 cores
    # [128, B]; slot s at [s%128, s//128]; bf16 exact for ints <= 255
    dltile = dlvals.reshape(-1, P).T.astype(ml_dtypes.bfloat16).copy()
    return wrapped, dltile


def build_host_data(x, edge_index, W1, b1, W2, b2, ncores=8, sg_size=7):
    N, IN = x.shape
    H = W1.shape[1]
    OUT = W2.shape[1]
    ngroups_abs = math.ceil(N / P)
    ngroups = math.ceil(ngroups_abs / ncores)
    if ngroups % 2:
        ngroups += 1  # even slot count so the AllGather splits in halves
    shard_pad = ngroups * P
    table_rows = shard_pad * ncores
    assert table_rows % NCHUNK == 0
    chunk = table_rows // NCHUNK
    assert chunk - 1 < 2**15, "chunk too large for int16 gather idx"

    dims = dict(
        N=N,
        IN=IN,
        H=H,
        OUT=OUT,
        ncores=ncores,
        ngroups=ngroups,
        shard_pad=shard_pad,
        table_rows=table_rows,
        chunk=chunk,
        sg_size=sg_size,
    )

    # Degrees include the self-loops, but the self-loop term itself is added
    # on-device via one identity matmul per dst group (agg[f,d] += own[d,f]
    # is a PE transpose) — gathering self rows would concentrate 128 edges of
    # one core into one (slot, chunk) segment and pad every other core.
    dst_e = np.asarray(edge_index[1])
    deg = np.bincount(
        np.concatenate([dst_e, np.arange(N)]), minlength=N
    )
    dis = 1.0 / np.sqrt(np.maximum(deg, 1.0))
    src = np.asarray(edge_index[0]).astype(np.int64)
    dst = dst_e.astype(np.int64)

    # Balanced group->-(core, slot) assignment: per-(slot, chunk) gather
    # segments are padded to the max over cores, so matching similar-sized
    # dst-groups in the same slot minimizes that padding.  Sort the 128-node
    # groups by edge count; rank r -> core r%ncores, slot r//ncores.
    gcount = np.bincount(dst // P, minlength=ngroups_abs)
    rank = np.empty(ngroups_abs, np.int64)
    rank[np.argsort(-gcount, kind="stable")] = np.arange(ngroups_abs)
    core_of = rank % ncores
    slot_of = rank // ncores

    gd = dst // P
    core = core_of[gd]
    eg = slot_of[gd]
    edl = (dst % P).astype(np.float32)

    # node n -> table row (same mapping for the x table and the h1 table).
    # Rows are laid out half-major: [half, core, slot_in_half, lane] so that
    # the inter-layer AllGather can run as two contiguous halves (the first
    # fires mid-layer-1 and hides under remaining compute).
    n_all = np.arange(N)
    gn = n_all // P
    nhalf = ngroups // 2
    halfrows = nhalf * P
    s_of = slot_of[gn]
    h_of = (s_of >= nhalf).astype(np.int64)
    noderow = (
        h_of * ncores * halfrows
        + core_of[gn] * halfrows
        + (s_of - h_of * nhalf) * P
        + (n_all % P)
    )
    dims["nhalf"] = nhalf
    trow = noderow[src]
    xt = np.zeros((table_rows, IN), ml_dtypes.bfloat16)
    xs = np.asarray(x, np.float32) * dis[:, None]
    xt[noderow] = xs.astype(ml_dtypes.bfloat16)

    ec = trow // chunk
    eloc = trow % chunk

    # both layers share the same (g, chunk) structure since table row mapping
    # is identical -> one schedule reused for both layers
    seg_len = np.zeros((ncores, ngroups, NCHUNK), np.int64)
    np.add.at(seg_len, (core, eg, ec), 1)
    schedule = make_schedule(dims, seg_len.max(axis=0))

    per_core = []
    for k in range(ncores):
        m = core == k
        wrapped, dltile = fill_core_slots(
            schedule, (eg[m], ec[m], eloc[m], edl[m]), dims
        )
        disn = np.zeros(shard_pad, np.float32)
        nm = core_of[gn] == k
        disn[slot_of[gn[nm]] * P + (n_all[nm] % P)] = dis[nm]
        dis_t = disn.reshape(ngroups, P).T.copy()  # [128, ngroups]
        halfrows = nhalf * P
        xselfk = np.vstack(
            [
                xt[k * halfrows : (k + 1) * halfrows],
                xt[(ncores + k) * halfrows : (ncores + k + 1) * halfrows],
            ]
        )
        per_core.append(dict(idx=wrapped, dl=dltile, dis=dis_t, xself=xselfk))

    dims["out_core"] = core_of[gn]
    dims["out_row"] = slot_of[gn] * P + (n_all % P)

    consts = dict(
        xt=xt,
        W1=np.asarray(W1, np.float32),
        W2=np.asarray(W2, np.float32),
        b1m=np.tile(np.asarray(b1, np.float32), (P, 1)),
        b2m=np.tile(np.asarray(b2, np.float32), (P, 1)),
        iota=np.tile(np.arange(P, dtype=ml_dtypes.bfloat16), (P, 1)),
        ident=np.eye(P, dtype=ml_dtypes.bfloat16),
    )
    return dims, schedule, consts, per_core


# -------------------------------------------------------------- device side


def build_kernel(nc, dims, schedule, variant="full"):
    dt = mybir.dt
    IN, H, OUT = dims["IN"], dims["H"], dims["OUT"]
    ncores = dims["ncores"]
    table_rows, chunk = dims["table_rows"], dims["chunk"]
    shard_pad = dims["shard_pad"]

    xt = nc.dram_tensor("xt", [table_rows, IN], dt.bfloat16, kind="ExternalInput")
    idx_in = nc.dram_tensor(
        "idx", [P, schedule["total_slots"] // 16], dt.int16, kind="ExternalInput"
    )
    dl_in = nc.dram_tensor(
        "dl", [P, schedule["total_batches"]], dt.bfloat16, kind="ExternalInput"
    )
    dis_in = nc.dram_tensor("dis", [P, dims["ngroups"]], dt.float32, kind="ExternalInput")
    W1_in = nc.dram_tensor("W1", [IN, H], dt.float32, kind="ExternalInput")
    W2_in = nc.dram_tensor("W2", [H, OUT], dt.float32, kind="ExternalInput")
    b1_in = nc.dram_tensor("b1m", [P, H], dt.float32, kind="ExternalInput")
    b2_in = nc.dram_tensor("b2m", [P, OUT], dt.float32, kind="ExternalInput")
    iota_in = nc.dram_tensor("iota", [P, P], dt.bfloat16, kind="ExternalInput")
    ident_in = nc.dram_tensor("ident", [P, P], dt.bfloat16, kind="ExternalInput")
    xself_in = nc.dram_tensor(
        "xself", [shard_pad, IN], dt.bfloat16, kind="ExternalInput"
    )

    h1self = nc.dram_tensor("h1self", [shard_pad, H], dt.bfloat16, kind="Internal")
    h1full = nc.dram_tensor(
        "h1full",
        [table_rows, H],
        dt.bfloat16,
        kind="Internal",
        addr_space="Shared" if ncores > 4 else "Local",
    )
    out = nc.dram_tensor("out", [shard_pad, OUT], dt.float32, kind="ExternalOutput")

    maxb = schedule["max_sg_batches"]

    from concourse.library_config import mlp as mlp_lib

    with tile.TileContext(nc) as tc:
        nc.gpsimd.load_library(mlp_lib)

        # One shared Pool register per distinct gather length (48-reg budget).
        regcache = {}

        def nidx_reg(v):
            if v not in regcache:
                r = nc.gpsimd.alloc_register(f"nidx{v}")
                nc.gpsimd.reg_mov(r, v)
                regcache[v] = r
            return regcache[v]
        with (
            tc.tile_pool(name="const", bufs=1) as cpool,
            tc.tile_pool(name="gather", bufs=2) as gpool,
            tc.tile_pool(name="meta", bufs=2) as mpool,
            tc.tile_pool(name="oh", bufs=2) as ohpool,
            tc.tile_pool(name="own", bufs=3) as opool,
            tc.tile_pool(name="ep", bufs=3) as epool,
            tc.tile_pool(name="aggp", bufs=2, space="PSUM") as aggpool,
            tc.tile_pool(name="densep", bufs=2, space="PSUM") as dpool,
        ):
            W1s = cpool.tile([IN, H], dt.float32)
            W2s = cpool.tile([H, OUT], dt.float32)
            b1s = cpool.tile([P, H], dt.float32)
            b2s = cpool.tile([P, OUT], dt.float32)
            iotas = cpool.tile([P, P], dt.bfloat16)
            idents = cpool.tile([P, P], dt.bfloat16)
            diss = cpool.tile([P, dims["ngroups"]], dt.float32)
            nc.sync.dma_start(out=W1s[:], in_=W1_in[:, :])
            nc.sync.dma_start(out=W2s[:], in_=W2_in[:, :])
            nc.sync.dma_start(out=b1s[:], in_=b1_in[:, :])
            nc.sync.dma_start(out=b2s[:], in_=b2_in[:, :])
            nc.sync.dma_start(out=iotas[:], in_=iota_in[:, :])
            nc.sync.dma_start(out=idents[:], in_=ident_in[:, :])
            nc.sync.dma_start(out=diss[:], in_=dis_in[:, :])

            c01 = cpool.tile([P, 1], dt.float32)
            nc.vector.memset(c01[:], 0.1)

            # Tile assigns SWDGE completion sems round-robin over 8 lanes
            # (one per Pool-DMA instruction, in program order) and each sem
            # is locked to a single SWDGE queue.  Derive queue from the same
            # mod-8 counter so lane L always sees one queue.  Each queue is
            # served by its own Q7 core pair, so gather pieces generate
            # descriptors concurrently on all four pairs.
            # Each piece is <= SUBMAX idxs so a single_packet stream stays
            # at <= 64 descriptors per DMA engine (the packet spec ceiling);
            # single-desc packets (single_packet=False) drain ~3x slower.
            qmap = [0, 1, 2, 3, 0, 1, 2, 3]
            qi = 0
            SUBMAX = 1024

            layers = (0,) if variant == "layer1" else (0, 1)
            for layer in layers:
                table = xt if layer == 0 else h1full
                HH = H if layer == 0 else OUT
                Wt = W1s if layer == 0 else W2s
                bt = b1s if layer == 0 else b2s

                nhalf = dims["nhalf"]
                halfrows = nhalf * P
                trig1 = -(-nhalf // dims["sg_size"]) - 1
                for si, s in enumerate(schedule["sgs"]):
                    nb = s["nbatches"]
                    gtile = gpool.tile([P, maxb * P], dt.bfloat16, tag="g")
                    itile = mpool.tile(
                        [P, schedule["max_sg_batches"] * 8], dt.int16, tag="i"
                    )
                    dtile = mpool.tile([P, maxb], dt.bfloat16, tag="d")
                    nc.sync.dma_start(
                        out=itile[:, : s["idx_ncol"]],
                        in_=idx_in[:, s["idx_col"] : s["idx_col"] + s["idx_ncol"]],
                    )
                    nc.sync.dma_start(
                        out=dtile[:, :nb],
                        in_=dl_in[:, s["batch_off"] : s["batch_off"] + nb],
                    )
                    # Issue pieces round-robin ACROSS chunk calls: the first
                    # groups' batches sit at the start of every chunk's
                    # segment, so interleaving lands their data first and the
                    # matmuls overlap the remaining descriptor generation.
                    pieces = []
                    for cnum, clen, coff, boff in s["calls"]:
                        for off in range(0, clen, SUBMAX):
                            if layer == 1 and si < 2:
                                # table half 0 (chunks 0-1) arrives first —
                                # gather it first while half 1 is in flight
                                key = (cnum // 2, off // SUBMAX, cnum)
                            else:
                                key = (off // SUBMAX, cnum, 0)
                            pieces.append((key, cnum, clen, coff, boff, off))
                    pieces.sort()
                    for _, cnum, clen, coff, boff, off in pieces:
                        plen = min(SUBMAX, clen - off)
                        o0 = boff * P + off
                        c0 = coff - s["idx_col"] + off // 16
                        nc.gpsimd.dma_gather(
                            out_ap=gtile[:, o0 : o0 + plen].rearrange(
                                "p (b f) -> p b f", f=P
                            ),
                            in_ap=table[cnum * chunk : (cnum + 1) * chunk, :],
                            idxs_ap=itile[:, c0 : c0 + plen // 16],
                            num_idxs=plen,
                            num_idxs_reg=nidx_reg(plen),
                            elem_size=IN if layer == 0 else H,
                            single_packet=True,
                            queue_num=qmap[qi % 8],
                        )
                        qi += 1
                    # One-hot matrix for every batch of this sg in one DVE op:
                    # oh[p, b, j] = (dl[p, b] == j).  tensor_tensor runs in
                    # single-port mode so it never locks GpSimd out of the
                    # shared SBUF port pair (tensor_scalar would).
                    ohtile = ohpool.tile([P, maxb * P], dt.bfloat16, tag="oh")
                    nc.vector.tensor_tensor(
                        out=ohtile[:, : nb * P].rearrange("p (b f) -> p b f", f=P),
                        in0=dtile[:, :nb].unsqueeze(2).broadcast_to([P, nb, P]),
                        in1=iotas[:, :].unsqueeze(1).broadcast_to([P, nb, P]),
                        op=mybir.AluOpType.is_equal,
                    )
                    # Self-loop term: agg[f, d] += own[d, f] — a PE transpose
                    # via an identity matmul in the group's PSUM chain.
                    for gg, bl in s["groups"]:
                        own = opool.tile([P, H], dt.bfloat16, tag="own")
                        ownsrc = xself_in if layer == 0 else h1self
                        nc.sync.dma_start(
                            out=own[:], in_=ownsrc[gg * P : (gg + 1) * P, :]
                        )
                        agg = aggpool.tile([P, P], dt.float32, tag="agg")
                        nc.tensor.matmul(
                            out=agg[:],
                            lhsT=own[:],
                            rhs=idents[:],
                            start=True,
                            stop=(len(bl) == 0),
                        )
                        for j, b in enumerate(bl):
                            nc.tensor.matmul(
                                out=agg[:],
                                lhsT=gtile[:, b * P : (b + 1) * P],
                                rhs=ohtile[:, b * P : (b + 1) * P],
                                start=False,
                                stop=(j == len(bl) - 1),
                            )
                        aggs = epool.tile([P, P], dt.float32, tag="aggs")
                        nc.vector.tensor_copy(out=aggs[:], in_=agg[:])
                        hraw = dpool.tile([P, HH], dt.float32, tag="hraw")
                        nc.tensor.matmul(
                            out=hraw[:], lhsT=aggs[:], rhs=Wt[:], start=True, stop=True
                        )
                        t1 = epool.tile([P, HH], dt.float32, tag="t1")
                        nc.vector.scalar_tensor_tensor(
                            out=t1[:],
                            in0=hraw[:],
                            scalar=diss[:, gg : gg + 1],
                            in1=bt[:],
                            op0=mybir.AluOpType.mult,
                            op1=mybir.AluOpType.add,
                        )
                        if layer == 0:
                            t2 = epool.tile([P, HH], dt.float32, tag="t2")
                            nc.scalar.activation(
                                out=t2[:], in_=t1[:], func=mybir.ActivationFunctionType.Relu
                            )
                            hst = epool.tile([P, HH], dt.bfloat16, tag="hst")
                            nc.vector.tensor_tensor(
                                out=hst[:],
                                in0=t2[:],
                                in1=diss[:, gg : gg + 1].broadcast_to([P, HH]),
                                op=mybir.AluOpType.mult,
                            )
                            nc.sync.dma_start(
                                out=h1self[gg * P : (gg + 1) * P, :], in_=hst[:]
                            )
                        else:
                            t2 = epool.tile([P, HH], dt.float32, tag="t2")
                            nc.scalar.activation(
                                out=t2[:],
                                in_=t1[:],
                                func=mybir.ActivationFunctionType.Sigmoid,
                            )
                            ot = epool.tile([P, HH], dt.float32, tag="ot")
                            nc.vector.scalar_tensor_tensor(
                                out=ot[:],
                                in0=t2[:],
                                scalar=0.8,
                                in1=c01[:, 0:1].broadcast_to([P, HH]),
                                op0=mybir.AluOpType.mult,
                                op1=mybir.AluOpType.add,
                            )
                            nc.sync.dma_start(
                                out=out[gg * P : (gg + 1) * P, :], in_=ot[:]
                            )
                    if layer == 0 and variant == "full" and si == trig1:
                        # slots [0, nhalf) are stored -> gather half 0 now so
                        # it hides under the remaining layer-1 compute
                        nc.gpsimd.collective_compute(
                            kind="AllGather",
                            op=mybir.AluOpType.bypass,
                            replica_groups=[list(range(ncores))],
                            ins=[h1self[:halfrows, :]],
                            outs=[h1full[: ncores * halfrows, :]],
                        )
                if layer == 0 and variant == "full":
                    nc.gpsimd.collective_compute(
                        kind="AllGather",
                        op=mybir.AluOpType.bypass,
                        replica_groups=[list(range(ncores))],
                        ins=[h1self[halfrows:, :]],
                        outs=[h1full[ncores * halfrows :, :]],
                    )
                elif layer == 0 and variant == "nocoll":
                    nc.sync.dma_start(out=h1full[:shard_pad, :], in_=h1self[:, :])
    return nc


def make_in_maps(dims, consts, per_core):
    in_maps = []
    for pc in per_core:
        in_maps.append(
            dict(
                xt=consts["xt"],
                idx=pc["idx"],
                dl=pc["dl"],
                dis=pc["dis"],
                xself=pc["xself"],
                W1=consts["W1"],
                W2=consts["W2"],
                b1m=consts["b1m"],
                b2m=consts["b2m"],
                iota=consts["iota"],
                ident=consts["ident"],
            )
        )
    return in_maps


def _install_ntff_hook():
    """Provide antenv.axon_hooks (missing on this image) so that
    run_bass_kernel_spmd(trace=True) can capture NTFF profiles via the
    axon .so's NRT-profile C ABI."""
    import sys
    import types

    if "antenv.axon_hooks" in sys.modules:
        return
    try:
        import antenv
        from trn_agent_boot.trn_boot import _ntff_profile_via_ctypes

        hook = _ntff_profile_via_ctypes("/opt/axon/libaxon_pjrt.so")
        mod = types.ModuleType("antenv.axon_hooks")
        mod._hook = hook

        def get_axon_ntff_profile_hook():
            return mod._hook

        def set_axon_ntff_profile_hook(h):
            mod._hook = h

        mod.get_axon_ntff_profile_hook = get_axon_ntff_profile_hook
        mod.set_axon_ntff_profile_hook = set_axon_ntff_profile_hook
        sys.modules["antenv.axon_hooks"] = mod
        antenv.axon_hooks = mod
    except Exception as e:  # pragma: no cover
        print("ntff hook install failed:", e)


def run(x, edge_index, W1, b1, W2, b2, ncores=8, sg_size=7, trace=False, variant="full"):
    from concourse import bass_utils

    if trace:
        _install_ntff_hook()

    dims, schedule, consts, per_core = build_host_data(
        x, edge_index, W1, b1, W2, b2, ncores=ncores, sg_size=sg_size
    )
    nc = bacc.Bacc(
        num_devices=ncores,
        num_swdge_queues=4,
        dynamic_dma_scratch_size=32768,
    )
    build_kernel(nc, dims, schedule, variant=variant)
    nc.compile()
    in_maps = make_in_maps(dims, consts, per_core)
    res = bass_utils.run_bass_kernel_spmd(
        nc, in_maps, core_ids=list(range(ncores)), trace=trace
    )
    allout = np.stack([r["out"] for r in res.results])  # [ncores, shard_pad, OUT]
    full = allout[dims["out_core"], dims["out_row"]]
    return full, res


# ------------------------------------------------------------- harness entry


def kernel(**inputs):
    """Full (unsharded) inputs -> full output, computed on 8 NeuronCores."""
    out, _ = run(
        np.asarray(inputs["x"], np.float32),
        np.asarray(inputs["edge_index"]),
        np.asarray(inputs["W1"], np.float32),
        np.asarray(inputs["b1"], np.float32),
        np.asarray(inputs["W2"], np.float32),
        np.asarray(inputs["b2"], np.float32),
        ncores=8,
        sg_size=7,
        trace=False,
    )
    return out.astype(np.float32)

